# revision 1
# baseline (speedup 1.0000x reference)
"""Dense GAT layer (attention + out-proj + residual + LayerNorm + SiLU + node mask)
as a fused Bass/Tile kernel on 8 Trainium2 NeuronCores.

Sharding: core = (b, half) with b = core//2, half = core%2. Each core computes
output rows [half*1024, (half+1)*1024) of batch b: it builds K/V for the full
2048 keys of its batch and Q only for its 1024 query rows, so no cross-core
reduction is needed — the host just concatenates row blocks.

Key structure (per core):
  * Loop order is query-chunk OUTER, head inner: all projections happen while
    streaming the first query chunk, and the output projection + LayerNorm for
    the first 512 rows overlaps the second chunk's attention (no serial tail).
  * S + adjacency mask fused in ONE fp8 DoubleRow matmul: slab0 = K^T.T@Q^T
    (d-contraction), slab1 = I.T@maskbias (adds 0 / -240 per pair). Both
    slabs are step-sliced views of two big SBUF tiles holding
    [qt_h..., maskbias_mc...] and [kt_(h,mc)..., ident]. exp of a -240-biased
    score underflows to zero, so no separate mask multiply exists anywhere.
  * softmax exp is split between ACT (hardware Exp -> fp8, scaled 1/8 via
    bias=-ln8) and DVE (Schraudolph: i16 = s*SCALE*128*log2e + (16256-384),
    bitcast bf16 == exp(s*SCALE)/8 with ~2% sawtooth that cancels in the row
    normalization).
  * A@V is DoubleRow fp8 (ACT groups) / classic bf16 (DVE groups), with a
    ones-column appended to V so O and the softmax row-sums come from the
    same accumulation.
  * SiLU uses Exp (1/(1+e^-x)) instead of Sigmoid so the whole kernel needs a
    single ACT function table (natural_log_exp) — no mid-kernel table loads.
node_mask only gates query rows (self-loops guarantee non-empty rows), so it
reduces to the final elementwise multiply.
"""

import math
from functools import lru_cache

import ml_dtypes
import numpy as np

import concourse.bacc as bacc
import concourse.mybir as mybir
import concourse.tile as tile
from concourse import masks

B, N, F = 4, 2048, 128
H, D = 8, 128
NQ = 1024
NCORES = 8
EPS = 1e-5
SCALE = 1.0 / math.sqrt(D)
LOG2E = 1.4426950408889634

F32 = mybir.dt.float32
F32R = mybir.dt.float32r
BF16 = mybir.dt.bfloat16
F8 = mybir.dt.float8e4
I16 = mybir.dt.int16
AF = mybir.ActivationFunctionType
ALU = mybir.AluOpType
DR = mybir.MatmulPerfMode.DoubleRow

NMC = N // 128   # 16 m-chunks
NCS = NQ // 512  # 2 query-column chunks
NG = 2           # m-chunks per group
NGRP = NMC // NG

DVE_GROUPS = (2, 5, 7)
ACT_CHUNKS = [mc for mc in range(NMC) if (mc // NG) not in DVE_GROUPS]
DVE_CHUNKS = [mc for mc in range(NMC) if (mc // NG) in DVE_GROUPS]
VA_SLOT = {mc: i for i, mc in enumerate(ACT_CHUNKS)}
VB_SLOT = {mc: i for i, mc in enumerate(DVE_CHUNKS)}

SCH_MUL = SCALE * 128.0 * LOG2E
SCH_ADD = 16256.0 - 384.0
EXP_BIAS = -math.log(8.0)

NSLOT_Q = H            # QM slots 0..7 = qt per head, 8..23 = maskbias chunks
NSLOT_QM = H + NMC
NSLOT_K = H * NMC      # K8 slots h*16+mc = kt, slot 128 = identity


def _build_program(affine: bool = False):
    nc = bacc.Bacc(
        "TRN2", target_bir_lowering=False, debug=False, num_devices=NCORES
    )
    d_xT = nc.declare_dram_parameter("xT", [F, N], F32R, isOutput=False)
    d_xqT = nc.declare_dram_parameter("xqT", [F, NQ], F32R, isOutput=False)
    d_xres = nc.declare_dram_parameter("xres", [128, 8, 128], F32, isOutput=False)
    d_maskb = nc.declare_dram_parameter("maskb", [128, NMC, NQ], F8, isOutput=False)
    d_wq = nc.declare_dram_parameter("wq", [F, H * D], F32R, isOutput=False)
    d_wk = nc.declare_dram_parameter("wk", [F, H * D], F32R, isOutput=False)
    d_wv = nc.declare_dram_parameter("wv", [F, H * D], F32R, isOutput=False)
    d_wo = nc.declare_dram_parameter("wo", [128, 8, 128], BF16, isOutput=False)
    d_gb = nc.declare_dram_parameter("gb", [2, 128], F32, isOutput=False)
    d_nm = nc.declare_dram_parameter("nm", [128, 8], F32, isOutput=False)
    d_out = nc.declare_dram_parameter("out", [128, 8, 128], F32, isOutput=True)

    with tile.TileContext(nc) as tc:
        with (
            tc.tile_pool(name="const", bufs=1) as const,
            tc.tile_pool(name="small", bufs=4) as sp,
        ):
            # DMA order: first S group needs wk, xT slice 0, wq, xqT, mask 0-1.
            wk = const.tile([128, H * D], F32R)
            nc.sync.dma_start(wk[:], d_wk[:])
            xT = const.tile([128, N], F32R)
            nc.sync.dma_start(xT[:, 0:512], d_xT[:, 0:512])
            wq = const.tile([128, H * D], F32R)
            nc.sync.dma_start(wq[:], d_wq[:])
            xqT = const.tile([128, NQ], F32R)
            nc.sync.dma_start(xqT[:], d_xqT[:])
            qm = const.tile([128, NSLOT_QM * NQ], F8, tag="qm")
            qmv = qm[:].rearrange("p (c n) -> p c n", n=NQ)

            def dma_mask(a, b):
                nc.sync.dma_start(qmv[:, H + a:H + b, :], d_maskb[:, a:b, :])

            dma_mask(0, 2)
            dma_mask(2, 4)
            wv = const.tile([128, H * D], F32R)
            nc.sync.dma_start(wv[:], d_wv[:])
            nc.sync.dma_start(xT[:, 512:1024], d_xT[:, 512:1024])
            dma_mask(4, 8)
            dma_mask(8, 12)
            nc.sync.dma_start(xT[:, 1024:1536], d_xT[:, 1024:1536])
            nc.sync.dma_start(xT[:, 1536:2048], d_xT[:, 1536:2048])
            dma_mask(12, 16)
            wo = const.tile([128, 8 * 128], BF16)
            wo_v = wo[:].rearrange("p (h d) -> p h d", d=128)
            nc.sync.dma_start(wo_v, d_wo[:])

            if affine:
                gbg = const.tile([1, 128], F32)
                nc.sync.dma_start(gbg[:], d_gb[0:1, :])
                gbb = const.tile([1, 128], F32)
                nc.sync.dma_start(gbb[:], d_gb[1:2, :])
            nm = const.tile([128, 8], F32)
            nc.sync.dma_start(nm[:], d_nm[:])
            xres = const.tile([128, 8 * 128], F32)
            xres_v = xres[:].rearrange("p (c d) -> p c d", d=128)
            nc.sync.dma_start(xres_v, d_xres[:])

            ident = const.tile([128, 128], BF16)
            masks.make_identity(nc, ident[:])
            k8 = const.tile([128, (NSLOT_K + 1) * 128], F8, tag="k8")
            k8v = k8[:].rearrange("p (c n) -> p c n", n=128)
            nc.vector.tensor_copy(k8v[:, NSLOT_K, :], ident[:])
            ones1 = const.tile([1, 128], F32)
            nc.vector.memset(ones1[:], 1.0)
            eps_t = const.tile([128, 1], F32)
            nc.vector.memset(eps_t[:], EPS)
            expb = const.tile([128, 1], F32)
            nc.vector.memset(expb[:], EXP_BIAS)
            one_col = const.tile([128, 1], F32)
            nc.vector.memset(one_col[:], 1.0)

            va8 = const.tile([128, len(ACT_CHUNKS) * 8 * 130], F8)
            va8_v = va8[:].rearrange("p (c h k) -> p c h k", h=8, k=130)
            vb16 = const.tile([128, len(DVE_CHUNKS) * 8 * 130], BF16)
            vb16_v = vb16[:].rearrange("p (c h k) -> p c h k", h=8, k=130)
            nc.vector.memset(va8_v[:, :, :, 128:129], 1.0)
            nc.vector.memset(vb16_v[:, :, :, 128:129], 1.0)

            OT = const.tile([128, H * NQ], BF16)  # O^T (unnormalized)
            OT_v = OT[:].rearrange("p (h n) -> p h n", n=NQ)

            if affine:
                gamma_bc = const.tile([128, 128], F32)
                beta_bc = const.tile([128, 128], F32)

            with (
                tc.tile_pool(name="hp", bufs=4) as hp,
                tc.tile_pool(name="pp", bufs=4) as pp,
                tc.tile_pool(name="ps_m", bufs=1, space="PSUM") as ps_m,
                tc.tile_pool(name="ps_s", bufs=2, space="PSUM") as ps_s,
                tc.tile_pool(name="ps_av", bufs=1, space="PSUM") as ps_av,
                tc.tile_pool(name="ps_o", bufs=1, space="PSUM") as ps_o,
            ):
                if affine:
                    gps = ps_m.tile([128, 512], F32, tag="pkq")
                    nc.tensor.matmul(gps[:, 0:128], ones1[:], gbg[:],
                                     start=True, stop=True)
                    nc.tensor.matmul(gps[:, 128:256], ones1[:], gbb[:],
                                     start=True, stop=True)
                    nc.vector.tensor_copy(gamma_bc[:], gps[:, 0:128])
                    nc.vector.tensor_copy(beta_bc[:], gps[:, 128:256])

                def emit_kproj(h, j):
                    pk = ps_m.tile([128, 512], F32, tag="pkq")
                    nc.tensor.matmul(pk[:], wk[:, h * 128:(h + 1) * 128],
                                     xT[:, j * 512:(j + 1) * 512],
                                     start=True, stop=True)
                    nc.scalar.copy(
                        k8[:, (h * NMC + j * 4) * 128:(h * NMC + j * 4 + 4) * 128],
                        pk[:])

                def emit_qproj(h, j):
                    pq = ps_m.tile([128, 512], F32, tag="pkq")
                    nc.tensor.matmul(pq[:], wq[:, h * 128:(h + 1) * 128],
                                     xqT[:, j * 512:(j + 1) * 512],
                                     start=True, stop=True)
                    nc.vector.tensor_copy(qmv[:, h, j * 512:(j + 1) * 512], pq[:])

                def emit_vproj(mc):
                    pv = ps_s.tile([128, 1024], F32, tag="sg")
                    # two matmuls: a single output may not cross a PSUM bank
                    for vj in range(2):
                        nc.tensor.matmul(pv[:, vj * 512:(vj + 1) * 512],
                                         xT[:, mc * 128:(mc + 1) * 128],
                                         wv[:, vj * 512:(vj + 1) * 512],
                                         start=True, stop=True)
                    pv_v = pv[:].rearrange("p (h d) -> p h d", d=128)
                    if mc in VA_SLOT:
                        dst = va8_v[:, VA_SLOT[mc], :, 0:128]
                    else:
                        dst = vb16_v[:, VB_SLOT[mc], :, 0:128]
                    if mc % 2 == 0:
                        nc.scalar.copy(dst, pv_v)
                    else:
                        nc.vector.tensor_copy(dst, pv_v)

                def emit_s(qc, h, g):
                    qsl = slice(qc * 512, (qc + 1) * 512)
                    sg = ps_s.tile([128, NG * 512], F32, tag="sg")
                    sg_v = sg[:].rearrange("p (c n) -> p c n", n=512)
                    for c in range(NG):
                        mc = g * NG + c
                        ks = h * NMC + mc
                        nc.tensor.matmul(
                            sg_v[:, c, :],
                            k8v[:, ks:NSLOT_K + 1:(NSLOT_K - ks), :],
                            qmv[:, h:H + mc + 1:(H + mc - h), qsl],
                            start=True, stop=True, perf_mode=DR,
                        )
                    return sg, sg_v

                def ln_half(hf, po, po_v):
                    c3 = [128, 4, 128]
                    cs = slice(hf * 4, (hf + 1) * 4)
                    fo = sp.tile([128, 4 * 128], F32, tag="fo")
                    fo_v = fo[:].rearrange("p (c d) -> p c d", d=128)
                    nc.vector.tensor_tensor(fo_v, po_v, xres_v[:, cs, :],
                                            ALU.add)
                    # E[x] on DVE in parallel with x^2 on ACT
                    mu = sp.tile([128, 4], F32, tag="mu")
                    nc.vector.tensor_reduce(mu[:], fo_v, mybir.AxisListType.X,
                                            ALU.add)
                    sq = sp.tile([128, 4 * 128], F32, tag="sq")
                    nc.scalar.square(sq[:], fo[:])
                    sq_v = sq[:].rearrange("p (c d) -> p c d", d=128)
                    vs = sp.tile([128, 4], F32, tag="vs")
                    nc.vector.tensor_reduce(vs[:], sq_v, mybir.AxisListType.X,
                                            ALU.add)
                    mean = sp.tile([128, 4], F32, tag="mean")
                    nc.vector.tensor_scalar_mul(mean[:], mu[:], 1.0 / 128.0)
                    msq = sp.tile([128, 4], F32, tag="msq")
                    nc.vector.tensor_tensor(msq[:], mean[:], mean[:], ALU.mult)
                    # var = E[x^2] - mean^2 + eps
                    ex2 = sp.tile([128, 4], F32, tag="ex2")
                    nc.vector.tensor_scalar(ex2[:], vs[:], 1.0 / 128.0, EPS,
                                            ALU.mult, ALU.add)
                    var = sp.tile([128, 4], F32, tag="var")
                    nc.vector.tensor_tensor(var[:], ex2[:], msq[:],
                                            ALU.subtract)
                    # rsqrt(var) via bf16 bit-trick seed + one Newton step
                    vb = sp.tile([128, 4], BF16, tag="vb")
                    nc.vector.tensor_copy(vb[:], var[:])
                    yi = sp.tile([128, 4], I16, tag="yi")
                    nc.vector.tensor_scalar(yi[:], vb[:].bitcast(I16), -0.5,
                                            24375.0, ALU.mult, ALU.add)
                    y0 = sp.tile([128, 4], F32, tag="y0")
                    nc.vector.tensor_copy(y0[:], yi[:].bitcast(BF16))
                    yy = sp.tile([128, 4], F32, tag="yy")
                    nc.vector.tensor_tensor(yy[:], y0[:], y0[:], ALU.mult)
                    vyy = sp.tile([128, 4], F32, tag="vyy")
                    nc.vector.tensor_tensor(vyy[:], var[:], yy[:], ALU.mult)
                    nwt = sp.tile([128, 4], F32, tag="nwt")
                    nc.vector.tensor_scalar(nwt[:], vyy[:], -0.5, 1.5,
                                            ALU.mult, ALU.add)
                    rs = sp.tile([128, 4], F32, tag="rs")
                    nc.vector.tensor_tensor(rs[:], y0[:], nwt[:], ALU.mult)
                    # fused normalize per chunk: nrm = fo*rs - (mean*rs)
                    mrs = sp.tile([128, 4], F32, tag="mrs")
                    nc.vector.tensor_tensor(mrs[:], mean[:], rs[:], ALU.mult)
                    nmrs = sp.tile([128, 4], F32, tag="nmrs")
                    nc.vector.tensor_scalar_mul(nmrs[:], mrs[:], -1.0)
                    nrm = sp.tile([128, 4 * 128], F32, tag="nrm")
                    nrm_v = nrm[:].rearrange("p (c d) -> p c d", d=128)
                    for c4 in range(4):
                        nc.vector.tensor_scalar(
                            nrm_v[:, c4, :], fo_v[:, c4, :],
                            rs[:, c4:c4 + 1], nmrs[:, c4:c4 + 1],
                            ALU.mult, ALU.add)
                    if affine:
                        g1 = sp.tile([128, 4 * 128], F32, tag="g1")
                        g1_v = g1[:].rearrange("p (c d) -> p c d", d=128)
                        nc.vector.tensor_tensor(
                            g1_v, nrm_v,
                            gamma_bc[:].unsqueeze(1).broadcast_to(c3), ALU.mult)
                        g2 = sp.tile([128, 4 * 128], F32, tag="g2")
                        g2_v = g2[:].rearrange("p (c d) -> p c d", d=128)
                        nc.vector.tensor_tensor(
                            g2_v, g1_v,
                            beta_bc[:].unsqueeze(1).broadcast_to(c3), ALU.add)
                    else:
                        g2, g2_v = nrm, nrm_v
                    gn = sp.tile([128, 4 * 128], F32, tag="gn")
                    gn_v = gn[:].rearrange("p (c d) -> p c d", d=128)
                    nc.vector.tensor_tensor(
                        gn_v, g2_v, nm[:, cs].unsqueeze(-1).broadcast_to(c3),
                        ALU.mult)
                    fin = sp.tile([128, 4 * 128], F32, tag="fin")
                    fin_v = fin[:].rearrange("p (c d) -> p c d", d=128)
                    if hf == 0:
                        # mid-kernel: SiLU via Exp so the ACT exp table stays
                        # loaded for the surrounding attention stream
                        ex = sp.tile([128, 4 * 128], F32, tag="ex")
                        nc.scalar.activation(ex[:], g2[:], AF.Exp, scale=-1.0)
                        ep = sp.tile([128, 4 * 128], F32, tag="ep")
                        nc.scalar.activation(ep[:], ex[:], AF.Identity,
                                             bias=one_col[:])
                        rc = sp.tile([128, 4 * 128], F32, tag="rc")
                        nc.vector.reciprocal(rc[:], ep[:])
                        nc.vector.tensor_tensor(fin_v, gn_v, rc[:].rearrange(
                            "p (c d) -> p c d", d=128), ALU.mult)
                    else:
                        # kernel end: real Sigmoid (table switch overlaps the
                        # preceding DVE chain; nothing needs exp afterwards)
                        sg2 = sp.tile([128, 4 * 128], F32, tag="sg2")
                        nc.scalar.activation(sg2[:], g2[:], AF.Sigmoid)
                        nc.vector.tensor_tensor(fin_v, gn_v, sg2[:].rearrange(
                            "p (c d) -> p c d", d=128), ALU.mult)
                    nc.sync.dma_start(d_out[:, cs, :], fin_v)

                # head-0: only the chunks the first S group needs go up
                # front; the rest slot into task 0 behind the S lookahead
                emit_kproj(0, 0)
                emit_qproj(0, 0)
                emit_qproj(0, 1)

                po_cur = [None, None]  # (tile, view) for the active qc

                def flush_block(blk):
                    # transpose the previous (qc, h)'s O block and fold it
                    # into the output projection incrementally; deferred so
                    # these PE ops sit behind the next head's S groups
                    # instead of stalling the exp stream.
                    fqc, fh, foh_v = blk
                    tpf = ps_m.tile([128, 512], F32, tag="pkq")
                    tp = tpf[:, 0:256].bitcast(BF16)
                    for s4 in range(4):
                        nc.tensor.matmul(
                            tp[:, s4 * 128:(s4 + 1) * 128],
                            foh_v[:, s4, :], ident[:],
                            is_transpose=True, start=True, stop=True,
                        )
                    otc = OT_v[:, fh, fqc * 512:(fqc + 1) * 512]
                    nc.vector.tensor_copy(otc, tp[:])
                    if fh == 0:
                        po = ps_o.tile([128, 512], F32, tag="po")
                        po_cur[0] = po
                        po_cur[1] = po[:].rearrange("p (c d) -> p c d", d=128)
                    po = po_cur[0]
                    for c4 in range(4):
                        c = fqc * 4 + c4
                        # start/stop are bank-granular: only the first/last
                        # matmul touching the bank may carry them.
                        nc.tensor.matmul(
                            po[:, c4 * 128:(c4 + 1) * 128],
                            OT_v[:, fh, c * 128:(c + 1) * 128],
                            wo_v[:, fh, :],
                            start=(fh == 0 and c4 == 0),
                            stop=(fh == H - 1 and c4 == 3),
                        )

                tasks = [(qc, h, g) for qc in range(NCS) for h in range(H)
                         for g in range(NGRP)]
                pend = emit_s(*tasks[0])
                deferred = None
                for i, (qc, h, g) in enumerate(tasks):
                    sg, sg_v = pend
                    is_dve = g in DVE_GROUPS
                    if is_dve:
                        if qc == 1 and h == H - 1:
                            # last head: DVE is winding down; ACT (idle at the
                            # tail) does the exp, still bf16 for the bf16 AV
                            pt16 = pp.tile([128, NG * 512], BF16, tag="pt16")
                            nc.scalar.activation(
                                pt16[:].rearrange("p (c n) -> p c n", n=512),
                                sg_v, AF.Exp, bias=expb[:], scale=SCALE)
                            praw_v = pt16[:].rearrange(
                                "p (c n) -> p c n", n=512)
                        else:
                            pti = pp.tile([128, NG * 512], I16, tag="pti")
                            nc.vector.tensor_scalar(pti[:], sg[:], SCH_MUL,
                                                    SCH_ADD, ALU.mult, ALU.add)
                            praw_v = pti[:].bitcast(BF16).rearrange(
                                "p (c n) -> p c n", n=512)
                    else:
                        pt8 = pp.tile([128, NG * 512], F8, tag="pt8")
                        pt8_v = pt8[:].rearrange("p (c n) -> p c n", n=512)
                        nc.scalar.activation(pt8_v, sg_v, AF.Exp,
                                             bias=expb[:], scale=SCALE)
                    # PE lookahead: next S group, then interleaved proj work
                    if i + 1 < len(tasks):
                        pend = emit_s(*tasks[i + 1])
                    if g == 1 and deferred is not None:
                        flush_block(deferred)
                        deferred = None
                        if qc == 1 and h == 0:
                            # half 0 is fully projected now; finalize it while
                            # qc1 attention streams
                            ln_half(0, po_cur[0], po_cur[1])
                    if qc == 0:
                        if h == 0:
                            if g == 0:
                                for j in range(1, 4):
                                    emit_kproj(0, j)
                            emit_vproj(g * 2)
                            emit_vproj(g * 2 + 1)
                        if h + 1 < H:
                            if 1 <= g <= 4:
                                emit_kproj(h + 1, g - 1)
                            if g == 2:
                                emit_qproj(h + 1, 0)
                            elif g == 3:
                                emit_qproj(h + 1, 1)
                    if g == 0:
                        avA = ps_av.tile([128, 512], F32, tag="avA")
                        avB = ps_av.tile([128, 512], F32, tag="avB")
                        oh = hp.tile([128, 4 * 128], BF16, tag="oh")
                        oh_v = oh[:].rearrange("p (s d) -> p s d", d=128)
                    if is_dve:
                        # c-outer: the first chunk's four AVs need only the
                        # first Schraudolph half
                        for c in range(NG):
                            mc = g * NG + c
                            for s in range(4):
                                av = avA if s < 2 else avB
                                off = (s % 2) * 256
                                nc.tensor.matmul(
                                    av[:, off:off + 129],
                                    praw_v[:, c, s * 128:(s + 1) * 128],
                                    vb16_v[:, VB_SLOT[mc], h, 0:129],
                                    start=(g == 0 and s % 2 == 0 and c == 0),
                                    stop=(g == NGRP - 1 and s % 2 == 1
                                          and c == NG - 1),
                                )
                    else:
                        for s in range(4):
                            av = avA if s < 2 else avB
                            off = (s % 2) * 256
                            slot = VA_SLOT[g * NG]
                            nc.tensor.matmul(
                                av[:, off:off + 129],
                                pt8_v[:, :, s * 128:(s + 1) * 128],
                                va8_v[:, slot:slot + 2, h, 0:129],
                                start=(g == 0 and s % 2 == 0),
                                stop=(g == NGRP - 1 and s % 2 == 1),
                                perf_mode=DR,
                            )
                    if g == NGRP - 1:
                        # normalize now (DVE only); transposes are deferred
                        for t_i, av in ((0, avA), (1, avB)):
                            av_v = av[:].rearrange("p (r q) -> p r q", q=256)
                            rec2 = sp.tile([128, 2], F32, tag="rec")
                            nc.vector.reciprocal(rec2[:].unsqueeze(-1),
                                                 av_v[:, :, 128:129])
                            nc.vector.tensor_tensor(
                                oh_v[:, 2 * t_i:2 * t_i + 2, :],
                                av_v[:, :, 0:128],
                                rec2[:].unsqueeze(-1).broadcast_to([128, 2, 128]),
                                ALU.mult)
                        deferred = (qc, h, oh_v)
                flush_block(deferred)
                ln_half(1, po_cur[0], po_cur[1])

    nc.compile()
    return nc


@lru_cache(maxsize=2)
def _program(affine: bool = False):
    return _build_program(affine)


class _Executor:
    """Caches the jitted shard_map executable across kernel() calls."""

    def __init__(self, nc):
        import jax
        import concourse.mybir as mb
        from concourse import bass2jax
        from jax.sharding import Mesh, PartitionSpec
        from jax.experimental.shard_map import shard_map

        bass2jax.install_neuronx_cc_hook()
        self.jax = jax
        partition_name = (
            nc.partition_id_tensor.name if nc.partition_id_tensor else None
        )
        in_names, out_names, out_avals, zero_shapes = [], [], [], []
        for alloc in nc.m.functions[0].allocations:
            if not isinstance(alloc, mb.MemoryLocationSet):
                continue
            name = alloc.memorylocations[0].name
            if alloc.kind == "ExternalInput":
                if name != partition_name:
                    in_names.append(name)
            elif alloc.kind == "ExternalOutput":
                out_names.append(name)
                shape = tuple(alloc.tensor_shape)
                dtype = mb.dt.np(alloc.dtype)
                out_avals.append(jax.core.ShapedArray(shape, dtype))
                zero_shapes.append((shape, dtype))
        self.n_params = len(in_names)
        self.in_names = list(in_names)
        self.out_names = out_names
        self.out_avals = out_avals
        self.zero_shapes = zero_shapes
        all_in = in_names + out_names + ([partition_name] if partition_name else [])
        donate = tuple(range(self.n_params, self.n_params + len(out_names)))

        def _body(*args):
            operands = list(args)
            if partition_name is not None:
                operands.append(bass2jax.partition_id_tensor())
            return tuple(bass2jax._bass_exec_p.bind(
                *operands,
                out_avals=tuple(out_avals),
                in_names=tuple(all_in),
                out_names=tuple(out_names),
                lowering_input_output_aliases=(),
                sim_require_finite=True,
                sim_require_nnan=True,
                nc=nc,
            ))

        devices = jax.devices()[:NCORES]
        mesh = Mesh(np.asarray(devices), ("core",))
        n_in = self.n_params + len(out_names)
        self.sharded = jax.jit(
            shard_map(_body, mesh=mesh,
                      in_specs=(PartitionSpec("core"),) * n_in,
                      out_specs=(PartitionSpec("core"),) * len(out_names),
                      check_rep=False),
            donate_argnums=donate, keep_unused=True,
        )

    def concat_inputs(self, in_maps):
        return [
            np.concatenate([np.asarray(m[name]) for m in in_maps], axis=0)
            for name in self.in_names
        ]

    def zeros(self):
        return [np.zeros((NCORES * s[0], *s[1:]), d) for s, d in self.zero_shapes]

    def run(self, concat_in):
        out_arrs = self.sharded(*concat_in, *self.zeros())
        return out_arrs

    def split(self, out_arrs):
        return [
            {name: np.asarray(out_arrs[i]).reshape(NCORES, *self.out_avals[i].shape)[c]
             for i, name in enumerate(self.out_names)}
            for c in range(NCORES)
        ]


@lru_cache(maxsize=2)
def _executor(affine: bool = False):
    return _Executor(_program(affine))


def _prep_core_inputs(core, x, attn_mask, node_mask, Wq, Wk, Wv, Wo, bo,
                      gamma, beta):
    b, half = core // 2, core % 2
    rsl = slice(half * NQ, (half + 1) * NQ)
    xb = np.ascontiguousarray(x[b])
    m = {}
    m["xT"] = np.ascontiguousarray(xb.T)
    m["xqT"] = np.ascontiguousarray(xb[rsl].T)
    m["xres"] = np.ascontiguousarray(
        (xb[rsl] + bo).reshape(8, 128, 128).transpose(1, 0, 2)
    )
    mT = np.where(attn_mask[b].T[:, rsl], 0.0, -240.0).astype(np.float32)
    m["maskb"] = np.ascontiguousarray(
        mT.reshape(NMC, 128, NQ).transpose(1, 0, 2)
    ).astype(ml_dtypes.float8_e4m3)
    m["wq"], m["wk"], m["wv"] = Wq, Wk, Wv
    m["wo"] = np.ascontiguousarray(
        Wo.reshape(8, 128, 128).transpose(1, 0, 2)).astype(ml_dtypes.bfloat16)
    m["gb"] = np.ascontiguousarray(np.stack([gamma, beta]))
    m["nm"] = np.ascontiguousarray(
        node_mask[b, rsl].astype(np.float32).reshape(8, 128).T
    )
    return m


def kernel(x, attn_mask, node_mask, Wq, Wk, Wv, Wo, bo, gamma, beta):
    x = np.asarray(x, np.float32)
    attn_mask = np.asarray(attn_mask, bool)
    node_mask = np.asarray(node_mask, bool)
    Wq = np.ascontiguousarray(np.asarray(Wq, np.float32))
    Wk = np.ascontiguousarray(np.asarray(Wk, np.float32))
    Wv = np.ascontiguousarray(np.asarray(Wv, np.float32))
    Wo = np.asarray(Wo, np.float32)
    bo = np.asarray(bo, np.float32)
    gamma = np.asarray(gamma, np.float32)
    beta = np.asarray(beta, np.float32)

    affine = not (np.all(gamma == 1.0) and np.all(beta == 0.0))
    ex = _executor(affine)
    in_maps = [
        _prep_core_inputs(c, x, attn_mask, node_mask, Wq, Wk, Wv, Wo, bo,
                          gamma, beta)
        for c in range(NCORES)
    ]
    results = ex.split(ex.run(ex.concat_inputs(in_maps)))
    out = np.empty((B, N, D), np.float32)
    for core in range(NCORES):
        b, half = core // 2, core % 2
        o = results[core]["out"]  # [128, 8, 128]
        out[b, half * NQ:(half + 1) * NQ] = (
            o.transpose(1, 0, 2).reshape(NQ, 128)
        )
    return out



# revision 85
# speedup vs baseline: 1.3865x; 1.3865x over previous
"""Dense GAT layer (attention + out-proj + residual + LayerNorm + SiLU + node mask)
as a fused Bass/Tile kernel on 8 Trainium2 NeuronCores.

Sharding: core = (b, half) with b = core//2, half = core%2. Each core computes
output rows [half*1024, (half+1)*1024) of batch b: it builds K/V for the full
2048 keys of its batch and Q only for its 1024 query rows, so no cross-core
reduction is needed — the host just concatenates row blocks.

Key structure (per core):
  * Loop order is query-chunk OUTER, head inner: all projections happen while
    streaming the first query chunk, and the output projection + LayerNorm for
    the first 512 rows overlaps the second chunk's attention (no serial tail).
  * The K and Q projections are folded algebraically: S_h = (x Wk_h)(xq
    Wq_h)^T = x (Wk_h Wq_h^T) xq^T, so the host ships the 128x128 per-head
    M_h^T = Wq_h Wk_h^T and the device computes only rq_h = M_h xq^T (8
    matmul pairs + 8 PSUM copies instead of K/Q projections with 48 copies).
  * S + adjacency mask fused in ONE fp8 DoubleRow matmul: slab0 = x8.T@rq
    (f-contraction against the raw fp8 input x), slab1 = I.T@maskbias (adds
    0 / -240 per pair). Both slabs are step-sliced views of two big SBUF
    tiles holding [rq_h..., maskbias_mc...] and [x8_mc..., ident]. exp of a
    -240-biased score underflows to zero, so no separate mask multiply
    exists anywhere.
  * softmax exp is split between ACT (hardware Exp -> fp8, scaled 1/8 via
    bias=-ln8) and DVE (Schraudolph: i16 = s*SCALE*128*log2e + (16256-384),
    bitcast bf16 == exp(s*SCALE)/8 with ~2% sawtooth that cancels in the row
    normalization).
  * A@V is DoubleRow fp8 (ACT groups) / classic bf16 (DVE groups), with a
    ones-column appended to V so O and the softmax row-sums come from the
    same accumulation.
  * SiLU uses Exp (1/(1+e^-x)) instead of Sigmoid so the whole kernel needs a
    single ACT function table (natural_log_exp) — no mid-kernel table loads.
node_mask only gates query rows (self-loops guarantee non-empty rows), so it
reduces to the final elementwise multiply.
"""

import math
from functools import lru_cache

import ml_dtypes
import numpy as np

import concourse.bacc as bacc
import concourse.mybir as mybir
import concourse.tile as tile
from concourse import masks

B, N, F = 4, 2048, 128
H, D = 8, 128
NQ = 1024
NCORES = 8
EPS = 1e-5
SCALE = 1.0 / math.sqrt(D)
LOG2E = 1.4426950408889634

F32 = mybir.dt.float32
F32R = mybir.dt.float32r
BF16 = mybir.dt.bfloat16
F8 = mybir.dt.float8e4
I16 = mybir.dt.int16
AF = mybir.ActivationFunctionType
ALU = mybir.AluOpType
DR = mybir.MatmulPerfMode.DoubleRow

NMC = N // 128   # 16 m-chunks
NCS = NQ // 512  # 2 query-column chunks
NG = 2           # m-chunks per group
NGRP = NMC // NG

# Per-qc exp-engine split. qc0 carries the projection copies, so ACT takes 5
# groups there. Groups whose engine differs between the phases keep BOTH V
# encodings (fp8 + bf16).
QC0_DVE = (2, 5, 7)
QC1_DVE = (2, 5, 7)
QC1_NORM_ACT = False
LAST_HEAD_ACT = False
LN_PIECES = 1
LN_STAGGER = False
DVE_Q = (frozenset(QC0_DVE), frozenset(QC1_DVE))
VA_GROUPS = [g for g in range(NGRP)
             if g not in DVE_Q[0] or g not in DVE_Q[1]]
VB_GROUPS = [g for g in range(NGRP) if g in DVE_Q[0] or g in DVE_Q[1]]
# fine rebalance: for SWING iterations, group SWING_G's second chunk does
# its exp on DVE (ACT runs hotter than DVE otherwise); its AV drops the DR
# pairing for those iterations (PE has headroom)
SWING_G = 0
SWING = frozenset([(1, 4), (1, 6), (1, 7)])
ACT_CHUNKS = [mc for mc in range(NMC) if (mc // NG) in VA_GROUPS]
DVE_CHUNKS = sorted(
    set(mc for mc in range(NMC) if (mc // NG) in VB_GROUPS)
    | {SWING_G * NG + 1})
VA_SLOT = {mc: i for i, mc in enumerate(ACT_CHUNKS)}
VB_SLOT = {mc: i for i, mc in enumerate(DVE_CHUNKS)}

SCH_MUL = SCALE * 128.0 * LOG2E
SCH_ADD = 16256.0 - 384.0
EXP_BIAS = -math.log(8.0)

NSLOT_Q = H            # QM slots 0..7 = rq per head, 8..23 = maskbias chunks
NSLOT_QM = H + NMC
NSLOT_X = NMC          # X8 slots 0..15 = fp8 x^T chunks, slot 16 = identity


def _build_program(affine: bool = False):
    nc = bacc.Bacc(
        "TRN2", target_bir_lowering=False, debug=False, num_devices=NCORES
    )
    d_xT8 = nc.declare_dram_parameter("xT8", [F, N], F8, isOutput=False)
    d_xqT = nc.declare_dram_parameter("xqT", [F, NQ], BF16, isOutput=False)
    d_xres = nc.declare_dram_parameter("xres", [128, 8, 128], F32, isOutput=False)
    d_maskb = nc.declare_dram_parameter("maskb", [128, NMC, NQ], F8, isOutput=False)
    # mT slab h = (Wk_h @ Wq_h^T)^T: scores fold the K and Q projections into
    # rq_h = M_h @ xq^T so S_h = x8^T @ rq8_h needs no on-device K or Q
    d_mT = nc.declare_dram_parameter("mT", [128, 8, 128], BF16, isOutput=False)
    # xa8/xb16: row-major x chunks with a baked ones column — A@x replaces
    # A@V since Wv folds into the output projection (wvo_h = Wv_h @ Wo_h)
    d_xa8 = nc.declare_dram_parameter(
        "xa8", [128, len(ACT_CHUNKS) * 130], F8, isOutput=False)
    d_xb16 = nc.declare_dram_parameter(
        "xb16", [128, len(DVE_CHUNKS) * 130], BF16, isOutput=False)
    d_wvo = nc.declare_dram_parameter("wvo", [128, 8, 128], BF16, isOutput=False)
    d_gb = nc.declare_dram_parameter("gb", [2, 128], F32, isOutput=False)
    d_nm = nc.declare_dram_parameter("nm", [128, 8], F32, isOutput=False)
    d_out = nc.declare_dram_parameter("out", [128, 8, 128], F32, isOutput=True)

    with tile.TileContext(nc) as tc:
        with (
            tc.tile_pool(name="const", bufs=1) as const,
            tc.tile_pool(name="small", bufs=4) as sp,
        ):
            # DMA order: first S group needs mT h0, xqT, x8 chunks 0-1,
            # mask 0-1; first AV group needs xa8 slots 0-1.
            xqT = const.tile([128, NQ], BF16)
            nc.sync.dma_start(xqT[:, 0:512], d_xqT[:, 0:512])
            mT = const.tile([128, 8 * 128], BF16)
            mT_v = mT[:].rearrange("p (h d) -> p h d", d=128)
            nc.sync.dma_start(mT_v, d_mT[:])
            x8k = const.tile([128, (NSLOT_X + 1) * 128], F8, tag="x8k")
            x8kv = x8k[:].rearrange("p (c n) -> p c n", n=128)
            nc.sync.dma_start(x8k[:, 0:512], d_xT8[:, 0:512])
            qm = const.tile([128, NSLOT_QM * NQ], F8, tag="qm")
            qmv = qm[:].rearrange("p (c n) -> p c n", n=NQ)

            def dma_mask(a, b):
                nc.sync.dma_start(qmv[:, H + a:H + b, :], d_maskb[:, a:b, :])

            dma_mask(0, 2)
            nc.sync.dma_start(xqT[:, 512:1024], d_xqT[:, 512:1024])
            dma_mask(2, 4)
            xa8 = const.tile([128, len(ACT_CHUNKS) * 130], F8)
            xa8_v = xa8[:].rearrange("p (c k) -> p c k", k=130)
            nc.sync.dma_start(xa8[:], d_xa8[:])
            xb16 = const.tile([128, len(DVE_CHUNKS) * 130], BF16)
            xb16_v = xb16[:].rearrange("p (c k) -> p c k", k=130)
            nc.sync.dma_start(xb16[:], d_xb16[:])
            nc.sync.dma_start(x8k[:, 512:2048], d_xT8[:, 512:2048])
            dma_mask(4, 8)
            dma_mask(8, 12)
            dma_mask(12, 16)
            wo = const.tile([128, 8 * 128], BF16)
            wo_v = wo[:].rearrange("p (h d) -> p h d", d=128)
            nc.sync.dma_start(wo_v, d_wvo[:])

            if affine:
                gbg = const.tile([1, 128], F32)
                nc.sync.dma_start(gbg[:], d_gb[0:1, :])
                gbb = const.tile([1, 128], F32)
                nc.sync.dma_start(gbb[:], d_gb[1:2, :])
            nm = const.tile([128, 8], F32)
            nc.sync.dma_start(nm[:], d_nm[:])
            xres = const.tile([128, 8 * 128], F32)
            xres_v = xres[:].rearrange("p (c d) -> p c d", d=128)
            nc.sync.dma_start(xres_v, d_xres[:])

            ident = const.tile([128, 128], BF16)
            masks.make_identity(nc, ident[:])
            nc.vector.tensor_copy(x8kv[:, NSLOT_X, :], ident[:])
            ones1 = const.tile([1, 128], F32)
            nc.vector.memset(ones1[:], 1.0)
            eps_t = const.tile([128, 1], F32)
            nc.vector.memset(eps_t[:], EPS)
            expb = const.tile([128, 1], F32)
            nc.vector.memset(expb[:], EXP_BIAS)
            one_col = const.tile([128, 1], F32)
            nc.vector.memset(one_col[:], 1.0)


            OT = const.tile([128, H * NQ], BF16)  # O^T (unnormalized)
            OT_v = OT[:].rearrange("p (h n) -> p h n", n=NQ)

            if affine:
                gamma_bc = const.tile([128, 128], F32)
                beta_bc = const.tile([128, 128], F32)

            with (
                tc.tile_pool(name="hp", bufs=4) as hp,
                tc.tile_pool(name="pp", bufs=4) as pp,
                tc.tile_pool(name="ps_s", bufs=5, space="PSUM") as ps_s,
                tc.tile_pool(name="ps_av", bufs=1, space="PSUM") as ps_av,
                tc.tile_pool(name="ps_o", bufs=1, space="PSUM") as ps_o,
            ):
                # PE warm-up: pe_busy_start latches on the first matmul, and
                # the p-state reaches full clock 3us later — fire a dummy now
                # (on ready-early ones1) so the first real matmuls run warm
                warm = ps_s.tile([128, 512], F32, tag="sg")
                nc.tensor.matmul(warm[:, 0:128], ones1[:], ones1[:],
                                 start=True, stop=True)
                if affine:
                    gps = ps_s.tile([128, 512], F32, tag="sg")
                    nc.tensor.matmul(gps[:, 0:128], ones1[:], gbg[:],
                                     start=True, stop=True)
                    nc.tensor.matmul(gps[:, 128:256], ones1[:], gbb[:],
                                     start=True, stop=True)
                    nc.vector.tensor_copy(gamma_bc[:], gps[:, 0:128])
                    nc.vector.tensor_copy(beta_bc[:], gps[:, 128:256])

                def emit_rq(h):
                    # rq_h = M_h @ xq^T: the K/Q projections folded into one
                    # [F, NQ] product against the host-prepped 128x128 M_h
                    for vj in range(2):
                        pv = ps_s.tile([128, 512], F32, tag="sg")
                        nc.tensor.matmul(pv[:],
                                         mT_v[:, h, :],
                                         xqT[:, vj * 512:(vj + 1) * 512],
                                         start=True, stop=True)
                        if vj == 0:
                            nc.scalar.copy(
                                qmv[:, h, vj * 512:(vj + 1) * 512], pv[:])
                        else:
                            nc.vector.tensor_copy(
                                qmv[:, h, vj * 512:(vj + 1) * 512], pv[:])

                def emit_s(qc, h, g):
                    # one 1-bank PSUM tile per chunk: the exp of chunk c
                    # releases its slot without waiting for the sibling, so
                    # the 4-slot ring advances half a group earlier than a
                    # 2-slot ring of whole groups would
                    qsl = slice(qc * 512, (qc + 1) * 512)
                    sgs = []
                    for c in range(NG):
                        mc = g * NG + c
                        sgc = ps_s.tile([128, 512], F32, tag="sg")
                        nc.tensor.matmul(
                            sgc[:],
                            x8kv[:, mc:NSLOT_X + 1:(NSLOT_X - mc), :],
                            qmv[:, h:H + mc + 1:(H + mc - h), qsl],
                            start=True, stop=True, perf_mode=DR,
                        )
                        sgs.append(sgc)
                    return sgs

                def ln_piece(hf, po_v, p2):
                    # one piece of the residual+LN+SiLU epilogue; split so
                    # pieces interleave with the surrounding streams
                    nch = 4 // LN_PIECES
                    c0 = hf * 4 + p2 * nch
                    c3 = [128, nch, 128]
                    cs = slice(c0, c0 + nch)
                    pcs = slice(p2 * nch, (p2 + 1) * nch)
                    fo = sp.tile([128, nch * 128], F32, tag=f"fo{p2}")
                    fo_v = fo[:].rearrange("p (c d) -> p c d", d=128)
                    nc.vector.tensor_tensor(fo_v, po_v[:, pcs, :],
                                            xres_v[:, cs, :], ALU.add)
                    # E[x] on DVE in parallel with x^2 on ACT
                    mu = sp.tile([128, nch], F32, tag=f"mu{p2}")
                    nc.vector.tensor_reduce(mu[:], fo_v, mybir.AxisListType.X,
                                            ALU.add)
                    sq = sp.tile([128, nch * 128], F32, tag=f"sq{p2}")
                    nc.scalar.square(sq[:], fo[:])
                    sq_v = sq[:].rearrange("p (c d) -> p c d", d=128)
                    vs = sp.tile([128, nch], F32, tag=f"vs{p2}")
                    nc.vector.tensor_reduce(vs[:], sq_v, mybir.AxisListType.X,
                                            ALU.add)
                    mean = sp.tile([128, nch], F32, tag=f"mean{p2}")
                    nc.vector.tensor_scalar_mul(mean[:], mu[:], 1.0 / 128.0)
                    msq = sp.tile([128, nch], F32, tag=f"msq{p2}")
                    nc.vector.tensor_tensor(msq[:], mean[:], mean[:], ALU.mult)
                    # var = E[x^2] - mean^2 + eps
                    ex2 = sp.tile([128, nch], F32, tag=f"ex2{p2}")
                    nc.vector.tensor_scalar(ex2[:], vs[:], 1.0 / 128.0, EPS,
                                            ALU.mult, ALU.add)
                    var = sp.tile([128, nch], F32, tag=f"var{p2}")
                    nc.vector.tensor_tensor(var[:], ex2[:], msq[:],
                                            ALU.subtract)
                    rs = sp.tile([128, nch], F32, tag=f"rs{p2}")
                    if hf == 1:
                        # tail: exact rsqrt = sqrt(1/var) — the 2-op chain
                        # beats the 7-op Newton ladder on the serial epilogue
                        rcv = sp.tile([128, nch], F32, tag=f"rcv{p2}")
                        nc.vector.reciprocal(rcv[:], var[:])
                        nc.scalar.activation(rs[:], rcv[:], AF.Sqrt)
                    else:
                        # rsqrt(var): bf16 bit-trick seed + one Newton step
                        vb = sp.tile([128, nch], BF16, tag=f"vb{p2}")
                        nc.vector.tensor_copy(vb[:], var[:])
                        yi = sp.tile([128, nch], I16, tag=f"yi{p2}")
                        nc.vector.tensor_scalar(yi[:], vb[:].bitcast(I16),
                                                -0.5, 24375.0,
                                                ALU.mult, ALU.add)
                        y0 = sp.tile([128, nch], F32, tag=f"y0{p2}")
                        nc.vector.tensor_copy(y0[:], yi[:].bitcast(BF16))
                        yy = sp.tile([128, nch], F32, tag=f"yy{p2}")
                        nc.vector.tensor_tensor(yy[:], y0[:], y0[:], ALU.mult)
                        vyy = sp.tile([128, nch], F32, tag=f"vyy{p2}")
                        nc.vector.tensor_tensor(vyy[:], var[:], yy[:],
                                                ALU.mult)
                        nwt = sp.tile([128, nch], F32, tag=f"nwt{p2}")
                        nc.vector.tensor_scalar(nwt[:], vyy[:], -0.5, 1.5,
                                                ALU.mult, ALU.add)
                        nc.vector.tensor_tensor(rs[:], y0[:], nwt[:],
                                                ALU.mult)
                    # fused normalize per chunk: nrm = fo*rs - (mean*rs)
                    mrs = sp.tile([128, nch], F32, tag=f"mrs{p2}")
                    nc.vector.tensor_tensor(mrs[:], mean[:], rs[:], ALU.mult)
                    nmrs = sp.tile([128, nch], F32, tag=f"nmrs{p2}")
                    nc.vector.tensor_scalar_mul(nmrs[:], mrs[:], -1.0)
                    nrm = sp.tile([128, nch * 128], F32, tag=f"nrm{p2}")
                    nrm_v = nrm[:].rearrange("p (c d) -> p c d", d=128)
                    for c4 in range(nch):
                        nc.vector.tensor_scalar(
                            nrm_v[:, c4, :], fo_v[:, c4, :],
                            rs[:, c4:c4 + 1], nmrs[:, c4:c4 + 1],
                            ALU.mult, ALU.add)
                    if affine:
                        g1 = sp.tile([128, nch * 128], F32, tag=f"g1{p2}")
                        g1_v = g1[:].rearrange("p (c d) -> p c d", d=128)
                        nc.vector.tensor_tensor(
                            g1_v, nrm_v,
                            gamma_bc[:].unsqueeze(1).broadcast_to(c3), ALU.mult)
                        g2 = sp.tile([128, nch * 128], F32, tag=f"g2{p2}")
                        g2_v = g2[:].rearrange("p (c d) -> p c d", d=128)
                        nc.vector.tensor_tensor(
                            g2_v, g1_v,
                            beta_bc[:].unsqueeze(1).broadcast_to(c3), ALU.add)
                    else:
                        g2, g2_v = nrm, nrm_v
                    gn = sp.tile([128, nch * 128], F32, tag=f"gn{p2}")
                    gn_v = gn[:].rearrange("p (c d) -> p c d", d=128)
                    nc.vector.tensor_tensor(
                        gn_v, g2_v, nm[:, cs].unsqueeze(-1).broadcast_to(c3),
                        ALU.mult)
                    fin = sp.tile([128, nch * 128], F32, tag=f"fin{p2}")
                    fin_v = fin[:].rearrange("p (c d) -> p c d", d=128)
                    if hf == 0:
                        # mid-kernel: SiLU via Exp so the ACT exp table stays
                        # loaded for the surrounding attention stream
                        ex = sp.tile([128, nch * 128], F32, tag=f"ex{p2}")
                        nc.scalar.activation(ex[:], g2[:], AF.Exp, scale=-1.0)
                        ep = sp.tile([128, nch * 128], F32, tag=f"ep{p2}")
                        nc.scalar.activation(ep[:], ex[:], AF.Identity,
                                             bias=one_col[:])
                        rc = sp.tile([128, nch * 128], F32, tag=f"rc{p2}")
                        nc.vector.reciprocal(rc[:], ep[:])
                        nc.vector.tensor_tensor(fin_v, gn_v, rc[:].rearrange(
                            "p (c d) -> p c d", d=128), ALU.mult)
                    else:
                        # kernel end: real Sigmoid (table switch overlaps the
                        # preceding DVE chain; nothing needs exp afterwards)
                        sg2 = sp.tile([128, nch * 128], F32, tag=f"sg2{p2}")
                        nc.scalar.activation(sg2[:], g2[:], AF.Sigmoid)
                        nc.vector.tensor_tensor(fin_v, gn_v, sg2[:].rearrange(
                            "p (c d) -> p c d", d=128), ALU.mult)
                    nc.sync.dma_start(d_out[:, cs, :], fin_v)

                def ln_half(hf, po, po_v):
                    for p2 in range(LN_PIECES):
                        ln_piece(hf, po_v, p2)

                # head-0 rq up front; later heads slot in behind the S
                # lookahead
                emit_rq(0)

                po_cur = [None, None]  # (tile, view) for the active qc

                def flush_block(blk):
                    # transpose the previous (qc, h)'s O block and fold it
                    # into the output projection incrementally; deferred so
                    # these PE ops sit behind the next head's S groups
                    # instead of stalling the exp stream.
                    fqc, fh, foh_v = blk
                    tpf = ps_s.tile([128, 512], F32, tag="sg")
                    tp = tpf[:, 0:256].bitcast(BF16)
                    for s4 in range(4):
                        nc.tensor.matmul(
                            tp[:, s4 * 128:(s4 + 1) * 128],
                            foh_v[:, s4, :], ident[:],
                            is_transpose=True, start=True, stop=True,
                        )
                    otc = OT_v[:, fh, fqc * 512:(fqc + 1) * 512]
                    nc.vector.tensor_copy(otc, tp[:])
                    if fh == 0:
                        po = ps_o.tile([128, 512], F32, tag="po")
                        po_cur[0] = po
                        po_cur[1] = po[:].rearrange("p (c d) -> p c d", d=128)
                    po = po_cur[0]
                    for c4 in range(4):
                        c = fqc * 4 + c4
                        # start/stop are bank-granular: only the first/last
                        # matmul touching the bank may carry them.
                        nc.tensor.matmul(
                            po[:, c4 * 128:(c4 + 1) * 128],
                            OT_v[:, fh, c * 128:(c + 1) * 128],
                            wo_v[:, fh, :],
                            start=(fh == 0 and c4 == 0),
                            stop=(fh == H - 1 and c4 == 3),
                        )

                tasks = [(qc, h, g) for qc in range(NCS) for h in range(H)
                         for g in range(NGRP)]
                pend = emit_s(*tasks[0])
                deferred = None
                for i, (qc, h, g) in enumerate(tasks):
                    sgs = pend
                    is_dve = g in DVE_Q[qc]
                    mixed = g == SWING_G and (qc, h) in SWING
                    if is_dve:
                        if LAST_HEAD_ACT and qc == 1 and h == H - 1:
                            # last head: DVE is winding down; ACT (idle at the
                            # tail) does the exp, still bf16 for the bf16 AV
                            pt16 = pp.tile([128, NG * 512], BF16, tag="pt16")
                            pt16_v = pt16[:].rearrange(
                                "p (c n) -> p c n", n=512)
                            for ec in range(NG):
                                nc.scalar.activation(
                                    pt16_v[:, ec, :], sgs[ec][:], AF.Exp,
                                    bias=expb[:], scale=SCALE)
                            praw_v = pt16_v
                        else:
                            pti = pp.tile([128, NG * 512], I16, tag="pti")
                            for ec in range(NG):
                                nc.vector.tensor_scalar(
                                    pti[:, ec * 512:(ec + 1) * 512],
                                    sgs[ec][:], SCH_MUL, SCH_ADD,
                                    ALU.mult, ALU.add)
                            praw_v = pti[:].bitcast(BF16).rearrange(
                                "p (c n) -> p c n", n=512)
                    elif mixed:
                        pt8 = pp.tile([128, NG * 512], F8, tag="pt8")
                        pt8_v = pt8[:].rearrange("p (c n) -> p c n", n=512)
                        nc.scalar.activation(pt8_v[:, 0, :], sgs[0][:],
                                             AF.Exp, bias=expb[:],
                                             scale=SCALE)
                        pti = pp.tile([128, NG * 512], I16, tag="pti")
                        nc.vector.tensor_scalar(pti[:, 0:512], sgs[1][:],
                                                SCH_MUL, SCH_ADD,
                                                ALU.mult, ALU.add)
                        praw1 = pti[:, 0:512].bitcast(BF16)
                    else:
                        pt8 = pp.tile([128, NG * 512], F8, tag="pt8")
                        pt8_v = pt8[:].rearrange("p (c n) -> p c n", n=512)
                        for ec in range(NG):
                            nc.scalar.activation(pt8_v[:, ec, :], sgs[ec][:],
                                                 AF.Exp, bias=expb[:],
                                                 scale=SCALE)
                    # PE lookahead: next S group, then interleaved proj work
                    if i + 1 < len(tasks):
                        pend = emit_s(*tasks[i + 1])
                    if g == 7 and deferred is not None:
                        flush_block(deferred)
                        deferred = None
                        if qc == 1 and h == 0:
                            # half 0 is fully projected now; finalize it while
                            # qc1 attention streams (piece 2 two tasks later
                            # so the DVE chain interleaves with Schraudolphs)
                            ln_piece(0, po_cur[1], 0)
                            if not LN_STAGGER:
                                for p2 in range(1, LN_PIECES):
                                    ln_piece(0, po_cur[1], p2)
                    if (LN_STAGGER and qc == 1 and h == 0 and g == 5
                            and LN_PIECES > 1):
                        ln_piece(0, po_cur[1], 1)
                    if qc == 0:
                        if h + 1 < H and g == 2:
                            emit_rq(h + 1)
                    if g == 0:
                        avA = ps_av.tile([128, 512], F32, tag="avA")
                        avB = ps_av.tile([128, 512], F32, tag="avB")
                        oh = hp.tile([128, 4 * 128], BF16, tag="oh")
                        oh_v = oh[:].rearrange("p (s d) -> p s d", d=128)
                    if is_dve:
                        # c-outer: the first chunk's four AVs need only the
                        # first Schraudolph half
                        for c in range(NG):
                            mc = g * NG + c
                            for s in range(4):
                                av = avA if s < 2 else avB
                                off = (s % 2) * 256
                                nc.tensor.matmul(
                                    av[:, off:off + 129],
                                    praw_v[:, c, s * 128:(s + 1) * 128],
                                    xb16_v[:, VB_SLOT[mc], 0:129],
                                    start=(g == 0 and s % 2 == 0 and c == 0),
                                    stop=(g == NGRP - 1 and s % 2 == 1
                                          and c == NG - 1),
                                )
                    elif mixed:
                        for s in range(4):
                            av = avA if s < 2 else avB
                            off = (s % 2) * 256
                            nc.tensor.matmul(
                                av[:, off:off + 129],
                                pt8_v[:, 0, s * 128:(s + 1) * 128],
                                xa8_v[:, VA_SLOT[g * NG], 0:129],
                                start=(g == 0 and s % 2 == 0),
                                stop=False,
                            )
                        for s in range(4):
                            av = avA if s < 2 else avB
                            off = (s % 2) * 256
                            nc.tensor.matmul(
                                av[:, off:off + 129],
                                praw1[:, s * 128:(s + 1) * 128],
                                xb16_v[:, VB_SLOT[g * NG + 1], 0:129],
                                start=False,
                                stop=(g == NGRP - 1 and s % 2 == 1),
                            )
                    else:
                        for s in range(4):
                            av = avA if s < 2 else avB
                            off = (s % 2) * 256
                            slot = VA_SLOT[g * NG]
                            nc.tensor.matmul(
                                av[:, off:off + 129],
                                pt8_v[:, :, s * 128:(s + 1) * 128],
                                xa8_v[:, slot:slot + 2, 0:129],
                                start=(g == 0 and s % 2 == 0),
                                stop=(g == NGRP - 1 and s % 2 == 1),
                                perf_mode=DR,
                            )
                    if g == NGRP - 1:
                        # normalize now; transposes are deferred. qc0: DVE
                        # (ACT is carrying the projection copies); qc1: ACT
                        # scale-activation (DVE carries 4 exp groups there)
                        for t_i, av in ((0, avA), (1, avB)):
                            av_v = av[:].rearrange("p (r q) -> p r q", q=256)
                            rec2 = sp.tile([128, 2], F32, tag="rec")
                            nc.vector.reciprocal(rec2[:].unsqueeze(-1),
                                                 av_v[:, :, 128:129])
                            if qc == 0 or not QC1_NORM_ACT:
                                nc.vector.tensor_tensor(
                                    oh_v[:, 2 * t_i:2 * t_i + 2, :],
                                    av_v[:, :, 0:128],
                                    rec2[:].unsqueeze(-1).broadcast_to(
                                        [128, 2, 128]),
                                    ALU.mult)
                            else:
                                for k2 in range(2):
                                    nc.scalar.activation(
                                        oh_v[:, 2 * t_i + k2, :],
                                        av_v[:, k2, 0:128], AF.Identity,
                                        scale=rec2[:, k2:k2 + 1])
                        deferred = (qc, h, oh_v)
                flush_block(deferred)
                ln_half(1, po_cur[0], po_cur[1])

    nc.compile()
    return nc


@lru_cache(maxsize=2)
def _program(affine: bool = False):
    return _build_program(affine)


class _Executor:
    """Caches the jitted shard_map executable across kernel() calls."""

    def __init__(self, nc):
        import jax
        import concourse.mybir as mb
        from concourse import bass2jax
        from jax.sharding import Mesh, PartitionSpec
        from jax.experimental.shard_map import shard_map

        bass2jax.install_neuronx_cc_hook()
        self.jax = jax
        partition_name = (
            nc.partition_id_tensor.name if nc.partition_id_tensor else None
        )
        in_names, out_names, out_avals, zero_shapes = [], [], [], []
        for alloc in nc.m.functions[0].allocations:
            if not isinstance(alloc, mb.MemoryLocationSet):
                continue
            name = alloc.memorylocations[0].name
            if alloc.kind == "ExternalInput":
                if name != partition_name:
                    in_names.append(name)
            elif alloc.kind == "ExternalOutput":
                out_names.append(name)
                shape = tuple(alloc.tensor_shape)
                dtype = mb.dt.np(alloc.dtype)
                out_avals.append(jax.core.ShapedArray(shape, dtype))
                zero_shapes.append((shape, dtype))
        self.n_params = len(in_names)
        self.in_names = list(in_names)
        self.out_names = out_names
        self.out_avals = out_avals
        self.zero_shapes = zero_shapes
        all_in = in_names + out_names + ([partition_name] if partition_name else [])
        donate = tuple(range(self.n_params, self.n_params + len(out_names)))

        def _body(*args):
            operands = list(args)
            if partition_name is not None:
                operands.append(bass2jax.partition_id_tensor())
            return tuple(bass2jax._bass_exec_p.bind(
                *operands,
                out_avals=tuple(out_avals),
                in_names=tuple(all_in),
                out_names=tuple(out_names),
                lowering_input_output_aliases=(),
                sim_require_finite=True,
                sim_require_nnan=True,
                nc=nc,
            ))

        devices = jax.devices()[:NCORES]
        mesh = Mesh(np.asarray(devices), ("core",))
        n_in = self.n_params + len(out_names)
        self.sharded = jax.jit(
            shard_map(_body, mesh=mesh,
                      in_specs=(PartitionSpec("core"),) * n_in,
                      out_specs=(PartitionSpec("core"),) * len(out_names),
                      check_rep=False),
            donate_argnums=donate, keep_unused=True,
        )

    def concat_inputs(self, in_maps):
        return [
            np.concatenate([np.asarray(m[name]) for m in in_maps], axis=0)
            for name in self.in_names
        ]

    def zeros(self):
        return [np.zeros((NCORES * s[0], *s[1:]), d) for s, d in self.zero_shapes]

    def run(self, concat_in):
        out_arrs = self.sharded(*concat_in, *self.zeros())
        return out_arrs

    def split(self, out_arrs):
        return [
            {name: np.asarray(out_arrs[i]).reshape(NCORES, *self.out_avals[i].shape)[c]
             for i, name in enumerate(self.out_names)}
            for c in range(NCORES)
        ]


@lru_cache(maxsize=2)
def _executor(affine: bool = False):
    return _Executor(_program(affine))


def _prep_core_inputs(core, x, attn_mask, node_mask, Wq, Wk, Wv, Wo, bo,
                      gamma, beta):
    b, half = core // 2, core % 2
    rsl = slice(half * NQ, (half + 1) * NQ)
    xb = np.ascontiguousarray(x[b])
    m = {}
    xbT = np.ascontiguousarray(xb.T)
    m["xT8"] = xbT.astype(ml_dtypes.float8_e4m3)
    # row-major x chunks with a ones column: A@x replaces A@V (Wv folded
    # into the output projection), and the ones give the softmax row-sums
    xR = np.ones((128, NMC, 130), np.float32)
    xR[:, :, 0:128] = xb.reshape(NMC, 128, F).transpose(1, 0, 2)
    xR[:, :, 129] = 0.0
    m["xa8"] = np.ascontiguousarray(
        xR[:, ACT_CHUNKS, :].reshape(128, -1)).astype(ml_dtypes.float8_e4m3)
    m["xb16"] = np.ascontiguousarray(
        xR[:, DVE_CHUNKS, :].reshape(128, -1)).astype(ml_dtypes.bfloat16)
    m["xqT"] = np.ascontiguousarray(xb[rsl].T).astype(ml_dtypes.bfloat16)
    m["xres"] = np.ascontiguousarray(
        (xb[rsl] + bo).reshape(8, 128, 128).transpose(1, 0, 2)
    )
    mT = np.where(attn_mask[b].T[:, rsl], 0.0, -240.0).astype(np.float32)
    m["maskb"] = np.ascontiguousarray(
        mT.reshape(NMC, 128, NQ).transpose(1, 0, 2)
    ).astype(ml_dtypes.float8_e4m3)
    # fold K/Q projections: mT slab h = Wq_h @ Wk_h^T, so that on device
    # rq_h = mT_h^T @ xq^T = (Wk_h Wq_h^T) xq^T and S_h = x^T . rq_h
    wq3 = Wq.reshape(F, H, D).transpose(1, 0, 2)   # [h, f, d]
    wk3 = Wk.reshape(F, H, D).transpose(1, 0, 2)
    mT3 = np.einsum("hfd,hgd->hfg", wq3, wk3)      # [h, f, g]
    m["mT"] = np.ascontiguousarray(
        mT3.transpose(1, 0, 2)).astype(ml_dtypes.bfloat16)  # [f, h, g]

    # fold Wv into the output projection: wvo_h = Wv_h @ Wo_h
    wv3 = Wv.reshape(F, H, D).transpose(1, 0, 2)          # [h, f, d]
    wo3 = Wo.reshape(H, D, 128)                           # [h, d, o]
    wvo3 = np.einsum("hfd,hdo->hfo", wv3, wo3)            # [h, f, o]
    m["wvo"] = np.ascontiguousarray(
        wvo3.transpose(1, 0, 2)).astype(ml_dtypes.bfloat16)  # [f, h, o]
    m["gb"] = np.ascontiguousarray(np.stack([gamma, beta]))
    m["nm"] = np.ascontiguousarray(
        node_mask[b, rsl].astype(np.float32).reshape(8, 128).T
    )
    return m


def kernel(x, attn_mask, node_mask, Wq, Wk, Wv, Wo, bo, gamma, beta):
    x = np.asarray(x, np.float32)
    attn_mask = np.asarray(attn_mask, bool)
    node_mask = np.asarray(node_mask, bool)
    Wq = np.ascontiguousarray(np.asarray(Wq, np.float32))
    Wk = np.ascontiguousarray(np.asarray(Wk, np.float32))
    Wv = np.ascontiguousarray(np.asarray(Wv, np.float32))
    Wo = np.asarray(Wo, np.float32)
    bo = np.asarray(bo, np.float32)
    gamma = np.asarray(gamma, np.float32)
    beta = np.asarray(beta, np.float32)

    affine = not (np.all(gamma == 1.0) and np.all(beta == 0.0))
    ex = _executor(affine)
    in_maps = [
        _prep_core_inputs(c, x, attn_mask, node_mask, Wq, Wk, Wv, Wo, bo,
                          gamma, beta)
        for c in range(NCORES)
    ]
    results = ex.split(ex.run(ex.concat_inputs(in_maps)))
    out = np.empty((B, N, D), np.float32)
    for core in range(NCORES):
        b, half = core // 2, core % 2
        o = results[core]["out"]  # [128, 8, 128]
        out[b, half * NQ:(half + 1) * NQ] = (
            o.transpose(1, 0, 2).reshape(NQ, 128)
        )
    return out



# revision 86
# speedup vs baseline: 1.3951x; 1.0062x over previous
"""Dense GAT layer (attention + out-proj + residual + LayerNorm + SiLU + node mask)
as a fused Bass/Tile kernel on 8 Trainium2 NeuronCores.

Sharding: core = (b, half) with b = core//2, half = core%2. Each core computes
output rows [half*1024, (half+1)*1024) of batch b: it builds K/V for the full
2048 keys of its batch and Q only for its 1024 query rows, so no cross-core
reduction is needed — the host just concatenates row blocks.

Key structure (per core):
  * Loop order is query-chunk OUTER, head inner: all projections happen while
    streaming the first query chunk, and the output projection + LayerNorm for
    the first 512 rows overlaps the second chunk's attention (no serial tail).
  * The K and Q projections are folded algebraically: S_h = (x Wk_h)(xq
    Wq_h)^T = x (Wk_h Wq_h^T) xq^T, so the host ships the 128x128 per-head
    M_h^T = Wq_h Wk_h^T and the device computes only rq_h = M_h xq^T (8
    matmul pairs + 8 PSUM copies instead of K/Q projections with 48 copies).
  * S + adjacency mask fused in ONE fp8 DoubleRow matmul: slab0 = x8.T@rq
    (f-contraction against the raw fp8 input x), slab1 = I.T@maskbias (adds
    0 / -240 per pair). Both slabs are step-sliced views of two big SBUF
    tiles holding [rq_h..., maskbias_mc...] and [x8_mc..., ident]. exp of a
    -240-biased score underflows to zero, so no separate mask multiply
    exists anywhere.
  * softmax exp is split between ACT (hardware Exp -> fp8, scaled 1/8 via
    bias=-ln8) and DVE (Schraudolph: i16 = s*SCALE*128*log2e + (16256-384),
    bitcast bf16 == exp(s*SCALE)/8 with ~2% sawtooth that cancels in the row
    normalization).
  * A@V is DoubleRow fp8 (ACT groups) / classic bf16 (DVE groups), with a
    ones-column appended to V so O and the softmax row-sums come from the
    same accumulation.
  * SiLU uses Exp (1/(1+e^-x)) instead of Sigmoid so the whole kernel needs a
    single ACT function table (natural_log_exp) — no mid-kernel table loads.
node_mask only gates query rows (self-loops guarantee non-empty rows), so it
reduces to the final elementwise multiply.
"""

import math
from functools import lru_cache

import ml_dtypes
import numpy as np

import concourse.bacc as bacc
import concourse.mybir as mybir
import concourse.tile as tile
from concourse import masks

B, N, F = 4, 2048, 128
H, D = 8, 128
NQ = 1024
NCORES = 8
EPS = 1e-5
SCALE = 1.0 / math.sqrt(D)
LOG2E = 1.4426950408889634

F32 = mybir.dt.float32
F32R = mybir.dt.float32r
BF16 = mybir.dt.bfloat16
F8 = mybir.dt.float8e4
I16 = mybir.dt.int16
AF = mybir.ActivationFunctionType
ALU = mybir.AluOpType
DR = mybir.MatmulPerfMode.DoubleRow

NMC = N // 128   # 16 m-chunks
NCS = NQ // 512  # 2 query-column chunks
NG = 2           # m-chunks per group
NGRP = NMC // NG

# Per-qc exp-engine split. qc0 carries the projection copies, so ACT takes 5
# groups there. Groups whose engine differs between the phases keep BOTH V
# encodings (fp8 + bf16).
QC0_DVE = (2, 5, 7)
QC1_DVE = (2, 5, 7)
QC1_NORM_ACT = False
LAST_HEAD_ACT = False
LN_PIECES = 1
LN_STAGGER = False
DVE_Q = (frozenset(QC0_DVE), frozenset(QC1_DVE))
VA_GROUPS = [g for g in range(NGRP)
             if g not in DVE_Q[0] or g not in DVE_Q[1]]
VB_GROUPS = [g for g in range(NGRP) if g in DVE_Q[0] or g in DVE_Q[1]]
# fine rebalance: for SWING iterations, group SWING_G's second chunk does
# its exp on DVE (ACT runs hotter than DVE otherwise); its AV drops the DR
# pairing for those iterations (PE has headroom)
SWING_G = 0
SWING = frozenset([(1, 4), (1, 6)])
ACT_CHUNKS = [mc for mc in range(NMC) if (mc // NG) in VA_GROUPS]
DVE_CHUNKS = sorted(
    set(mc for mc in range(NMC) if (mc // NG) in VB_GROUPS)
    | {SWING_G * NG + 1})
VA_SLOT = {mc: i for i, mc in enumerate(ACT_CHUNKS)}
VB_SLOT = {mc: i for i, mc in enumerate(DVE_CHUNKS)}

SCH_MUL = SCALE * 128.0 * LOG2E
SCH_ADD = 16256.0 - 384.0
EXP_BIAS = -math.log(8.0)

NSLOT_Q = H            # QM slots 0..7 = rq per head, 8..23 = maskbias chunks
NSLOT_QM = H + NMC
NSLOT_X = NMC          # X8 slots 0..15 = fp8 x^T chunks, slot 16 = identity


def _build_program(affine: bool = False):
    nc = bacc.Bacc(
        "TRN2", target_bir_lowering=False, debug=False, num_devices=NCORES
    )
    d_xT8 = nc.declare_dram_parameter("xT8", [F, N], F8, isOutput=False)
    d_xqT = nc.declare_dram_parameter("xqT", [F, NQ], BF16, isOutput=False)
    d_xres = nc.declare_dram_parameter("xres", [128, 8, 128], F32, isOutput=False)
    d_maskb = nc.declare_dram_parameter("maskb", [128, NMC, NQ], F8, isOutput=False)
    # mT slab h = (Wk_h @ Wq_h^T)^T: scores fold the K and Q projections into
    # rq_h = M_h @ xq^T so S_h = x8^T @ rq8_h needs no on-device K or Q
    d_mT = nc.declare_dram_parameter("mT", [128, 8, 128], BF16, isOutput=False)
    # xa8/xb16: row-major x chunks with a baked ones column — A@x replaces
    # A@V since Wv folds into the output projection (wvo_h = Wv_h @ Wo_h)
    d_xa8 = nc.declare_dram_parameter(
        "xa8", [128, len(ACT_CHUNKS) * 130], F8, isOutput=False)
    d_xb16 = nc.declare_dram_parameter(
        "xb16", [128, len(DVE_CHUNKS) * 130], BF16, isOutput=False)
    d_wvo = nc.declare_dram_parameter("wvo", [128, 8, 128], BF16, isOutput=False)
    d_gb = nc.declare_dram_parameter("gb", [2, 128], F32, isOutput=False)
    d_nm = nc.declare_dram_parameter("nm", [128, 8], F32, isOutput=False)
    d_out = nc.declare_dram_parameter("out", [128, 8, 128], F32, isOutput=True)

    with tile.TileContext(nc) as tc:
        with (
            tc.tile_pool(name="const", bufs=1) as const,
            tc.tile_pool(name="small", bufs=4) as sp,
        ):
            # DMA order: first S group needs mT h0, xqT, x8 chunks 0-1,
            # mask 0-1; first AV group needs xa8 slots 0-1.
            xqT = const.tile([128, NQ], BF16)
            nc.sync.dma_start(xqT[:, 0:512], d_xqT[:, 0:512])
            mT = const.tile([128, 8 * 128], BF16)
            mT_v = mT[:].rearrange("p (h d) -> p h d", d=128)
            nc.sync.dma_start(mT_v, d_mT[:])
            x8k = const.tile([128, (NSLOT_X + 1) * 128], F8, tag="x8k")
            x8kv = x8k[:].rearrange("p (c n) -> p c n", n=128)
            nc.sync.dma_start(x8k[:, 0:512], d_xT8[:, 0:512])
            qm = const.tile([128, NSLOT_QM * NQ], F8, tag="qm")
            qmv = qm[:].rearrange("p (c n) -> p c n", n=NQ)

            def dma_mask(a, b):
                nc.sync.dma_start(qmv[:, H + a:H + b, :], d_maskb[:, a:b, :])

            dma_mask(0, 2)
            nc.sync.dma_start(xqT[:, 512:1024], d_xqT[:, 512:1024])
            dma_mask(2, 4)
            xa8 = const.tile([128, len(ACT_CHUNKS) * 130], F8)
            xa8_v = xa8[:].rearrange("p (c k) -> p c k", k=130)
            nc.sync.dma_start(xa8[:], d_xa8[:])
            xb16 = const.tile([128, len(DVE_CHUNKS) * 130], BF16)
            xb16_v = xb16[:].rearrange("p (c k) -> p c k", k=130)
            nc.sync.dma_start(xb16[:], d_xb16[:])
            nc.sync.dma_start(x8k[:, 512:2048], d_xT8[:, 512:2048])
            dma_mask(4, 8)
            dma_mask(8, 12)
            dma_mask(12, 16)
            wo = const.tile([128, 8 * 128], BF16)
            wo_v = wo[:].rearrange("p (h d) -> p h d", d=128)
            nc.sync.dma_start(wo_v, d_wvo[:])

            if affine:
                gbg = const.tile([1, 128], F32)
                nc.sync.dma_start(gbg[:], d_gb[0:1, :])
                gbb = const.tile([1, 128], F32)
                nc.sync.dma_start(gbb[:], d_gb[1:2, :])
            nm = const.tile([128, 8], F32)
            nc.sync.dma_start(nm[:], d_nm[:])
            xres = const.tile([128, 8 * 128], F32)
            xres_v = xres[:].rearrange("p (c d) -> p c d", d=128)
            nc.sync.dma_start(xres_v, d_xres[:])

            ident = const.tile([128, 128], BF16)
            masks.make_identity(nc, ident[:])
            nc.vector.tensor_copy(x8kv[:, NSLOT_X, :], ident[:])
            ones1 = const.tile([1, 128], F32)
            nc.vector.memset(ones1[:], 1.0)
            eps_t = const.tile([128, 1], F32)
            nc.vector.memset(eps_t[:], EPS)
            expb = const.tile([128, 1], F32)
            nc.vector.memset(expb[:], EXP_BIAS)
            one_col = const.tile([128, 1], F32)
            nc.vector.memset(one_col[:], 1.0)


            OT = const.tile([128, H * NQ], BF16)  # O^T (unnormalized)
            OT_v = OT[:].rearrange("p (h n) -> p h n", n=NQ)

            if affine:
                gamma_bc = const.tile([128, 128], F32)
                beta_bc = const.tile([128, 128], F32)

            with (
                tc.tile_pool(name="hp", bufs=4) as hp,
                tc.tile_pool(name="pp", bufs=4) as pp,
                tc.tile_pool(name="ps_s", bufs=5, space="PSUM") as ps_s,
                tc.tile_pool(name="ps_av", bufs=1, space="PSUM") as ps_av,
                tc.tile_pool(name="ps_o", bufs=1, space="PSUM") as ps_o,
            ):
                # PE warm-up: pe_busy_start latches on the first matmul, and
                # the p-state reaches full clock 3us later — fire a dummy now
                # (on ready-early ones1) so the first real matmuls run warm
                warm = ps_s.tile([128, 512], F32, tag="sg")
                nc.tensor.matmul(warm[:, 0:128], ones1[:], ones1[:],
                                 start=True, stop=True)
                if affine:
                    gps = ps_s.tile([128, 512], F32, tag="sg")
                    nc.tensor.matmul(gps[:, 0:128], ones1[:], gbg[:],
                                     start=True, stop=True)
                    nc.tensor.matmul(gps[:, 128:256], ones1[:], gbb[:],
                                     start=True, stop=True)
                    nc.vector.tensor_copy(gamma_bc[:], gps[:, 0:128])
                    nc.vector.tensor_copy(beta_bc[:], gps[:, 128:256])

                def emit_rq(h):
                    # rq_h = M_h @ xq^T: the K/Q projections folded into one
                    # [F, NQ] product against the host-prepped 128x128 M_h
                    for vj in range(2):
                        pv = ps_s.tile([128, 512], F32, tag="sg")
                        nc.tensor.matmul(pv[:],
                                         mT_v[:, h, :],
                                         xqT[:, vj * 512:(vj + 1) * 512],
                                         start=True, stop=True)
                        if vj == 0:
                            nc.scalar.copy(
                                qmv[:, h, vj * 512:(vj + 1) * 512], pv[:])
                        else:
                            nc.vector.tensor_copy(
                                qmv[:, h, vj * 512:(vj + 1) * 512], pv[:])

                def emit_s(qc, h, g):
                    # one 1-bank PSUM tile per chunk: the exp of chunk c
                    # releases its slot without waiting for the sibling, so
                    # the 4-slot ring advances half a group earlier than a
                    # 2-slot ring of whole groups would
                    qsl = slice(qc * 512, (qc + 1) * 512)
                    sgs = []
                    for c in range(NG):
                        mc = g * NG + c
                        sgc = ps_s.tile([128, 512], F32, tag="sg")
                        nc.tensor.matmul(
                            sgc[:],
                            x8kv[:, mc:NSLOT_X + 1:(NSLOT_X - mc), :],
                            qmv[:, h:H + mc + 1:(H + mc - h), qsl],
                            start=True, stop=True, perf_mode=DR,
                        )
                        sgs.append(sgc)
                    return sgs

                def ln_piece(hf, po_v, p2):
                    # one piece of the residual+LN+SiLU epilogue; split so
                    # pieces interleave with the surrounding streams
                    nch = 4 // LN_PIECES
                    c0 = hf * 4 + p2 * nch
                    c3 = [128, nch, 128]
                    cs = slice(c0, c0 + nch)
                    pcs = slice(p2 * nch, (p2 + 1) * nch)
                    fo = sp.tile([128, nch * 128], F32, tag=f"fo{p2}")
                    fo_v = fo[:].rearrange("p (c d) -> p c d", d=128)
                    nc.vector.tensor_tensor(fo_v, po_v[:, pcs, :],
                                            xres_v[:, cs, :], ALU.add)
                    # E[x] on DVE in parallel with x^2 on ACT
                    mu = sp.tile([128, nch], F32, tag=f"mu{p2}")
                    nc.vector.tensor_reduce(mu[:], fo_v, mybir.AxisListType.X,
                                            ALU.add)
                    sq = sp.tile([128, nch * 128], F32, tag=f"sq{p2}")
                    nc.scalar.square(sq[:], fo[:])
                    sq_v = sq[:].rearrange("p (c d) -> p c d", d=128)
                    vs = sp.tile([128, nch], F32, tag=f"vs{p2}")
                    nc.vector.tensor_reduce(vs[:], sq_v, mybir.AxisListType.X,
                                            ALU.add)
                    mean = sp.tile([128, nch], F32, tag=f"mean{p2}")
                    nc.vector.tensor_scalar_mul(mean[:], mu[:], 1.0 / 128.0)
                    msq = sp.tile([128, nch], F32, tag=f"msq{p2}")
                    nc.vector.tensor_tensor(msq[:], mean[:], mean[:], ALU.mult)
                    # var = E[x^2] - mean^2 + eps
                    ex2 = sp.tile([128, nch], F32, tag=f"ex2{p2}")
                    nc.vector.tensor_scalar(ex2[:], vs[:], 1.0 / 128.0, EPS,
                                            ALU.mult, ALU.add)
                    var = sp.tile([128, nch], F32, tag=f"var{p2}")
                    nc.vector.tensor_tensor(var[:], ex2[:], msq[:],
                                            ALU.subtract)
                    rs = sp.tile([128, nch], F32, tag=f"rs{p2}")
                    if hf == 1:
                        # tail: exact rsqrt = sqrt(1/var) — the 2-op chain
                        # beats the 7-op Newton ladder on the serial epilogue
                        rcv = sp.tile([128, nch], F32, tag=f"rcv{p2}")
                        nc.vector.reciprocal(rcv[:], var[:])
                        nc.scalar.activation(rs[:], rcv[:], AF.Sqrt)
                    else:
                        # rsqrt(var): bf16 bit-trick seed + one Newton step
                        vb = sp.tile([128, nch], BF16, tag=f"vb{p2}")
                        nc.vector.tensor_copy(vb[:], var[:])
                        yi = sp.tile([128, nch], I16, tag=f"yi{p2}")
                        nc.vector.tensor_scalar(yi[:], vb[:].bitcast(I16),
                                                -0.5, 24375.0,
                                                ALU.mult, ALU.add)
                        y0 = sp.tile([128, nch], F32, tag=f"y0{p2}")
                        nc.vector.tensor_copy(y0[:], yi[:].bitcast(BF16))
                        yy = sp.tile([128, nch], F32, tag=f"yy{p2}")
                        nc.vector.tensor_tensor(yy[:], y0[:], y0[:], ALU.mult)
                        vyy = sp.tile([128, nch], F32, tag=f"vyy{p2}")
                        nc.vector.tensor_tensor(vyy[:], var[:], yy[:],
                                                ALU.mult)
                        nwt = sp.tile([128, nch], F32, tag=f"nwt{p2}")
                        nc.vector.tensor_scalar(nwt[:], vyy[:], -0.5, 1.5,
                                                ALU.mult, ALU.add)
                        nc.vector.tensor_tensor(rs[:], y0[:], nwt[:],
                                                ALU.mult)
                    # fused normalize per chunk: nrm = fo*rs - (mean*rs)
                    mrs = sp.tile([128, nch], F32, tag=f"mrs{p2}")
                    nc.vector.tensor_tensor(mrs[:], mean[:], rs[:], ALU.mult)
                    nmrs = sp.tile([128, nch], F32, tag=f"nmrs{p2}")
                    nc.vector.tensor_scalar_mul(nmrs[:], mrs[:], -1.0)
                    nrm = sp.tile([128, nch * 128], F32, tag=f"nrm{p2}")
                    nrm_v = nrm[:].rearrange("p (c d) -> p c d", d=128)
                    for c4 in range(nch):
                        nc.vector.tensor_scalar(
                            nrm_v[:, c4, :], fo_v[:, c4, :],
                            rs[:, c4:c4 + 1], nmrs[:, c4:c4 + 1],
                            ALU.mult, ALU.add)
                    if affine:
                        g1 = sp.tile([128, nch * 128], F32, tag=f"g1{p2}")
                        g1_v = g1[:].rearrange("p (c d) -> p c d", d=128)
                        nc.vector.tensor_tensor(
                            g1_v, nrm_v,
                            gamma_bc[:].unsqueeze(1).broadcast_to(c3), ALU.mult)
                        g2 = sp.tile([128, nch * 128], F32, tag=f"g2{p2}")
                        g2_v = g2[:].rearrange("p (c d) -> p c d", d=128)
                        nc.vector.tensor_tensor(
                            g2_v, g1_v,
                            beta_bc[:].unsqueeze(1).broadcast_to(c3), ALU.add)
                    else:
                        g2, g2_v = nrm, nrm_v
                    gn = sp.tile([128, nch * 128], F32, tag=f"gn{p2}")
                    gn_v = gn[:].rearrange("p (c d) -> p c d", d=128)
                    nc.vector.tensor_tensor(
                        gn_v, g2_v, nm[:, cs].unsqueeze(-1).broadcast_to(c3),
                        ALU.mult)
                    fin = sp.tile([128, nch * 128], F32, tag=f"fin{p2}")
                    fin_v = fin[:].rearrange("p (c d) -> p c d", d=128)
                    if hf == 0:
                        # mid-kernel: SiLU via Exp so the ACT exp table stays
                        # loaded for the surrounding attention stream
                        ex = sp.tile([128, nch * 128], F32, tag=f"ex{p2}")
                        nc.scalar.activation(ex[:], g2[:], AF.Exp, scale=-1.0)
                        ep = sp.tile([128, nch * 128], F32, tag=f"ep{p2}")
                        nc.scalar.activation(ep[:], ex[:], AF.Identity,
                                             bias=one_col[:])
                        rc = sp.tile([128, nch * 128], F32, tag=f"rc{p2}")
                        nc.vector.reciprocal(rc[:], ep[:])
                        nc.vector.tensor_tensor(fin_v, gn_v, rc[:].rearrange(
                            "p (c d) -> p c d", d=128), ALU.mult)
                    else:
                        # kernel end: real Sigmoid (table switch overlaps the
                        # preceding DVE chain; nothing needs exp afterwards)
                        sg2 = sp.tile([128, nch * 128], F32, tag=f"sg2{p2}")
                        nc.scalar.activation(sg2[:], g2[:], AF.Sigmoid)
                        nc.vector.tensor_tensor(fin_v, gn_v, sg2[:].rearrange(
                            "p (c d) -> p c d", d=128), ALU.mult)
                    nc.sync.dma_start(d_out[:, cs, :], fin_v)

                def ln_half(hf, po, po_v):
                    for p2 in range(LN_PIECES):
                        ln_piece(hf, po_v, p2)

                # head-0 rq up front; later heads slot in behind the S
                # lookahead
                emit_rq(0)

                po_cur = [None, None]  # (tile, view) for the active qc

                def flush_block(blk):
                    # transpose the previous (qc, h)'s O block and fold it
                    # into the output projection incrementally; deferred so
                    # these PE ops sit behind the next head's S groups
                    # instead of stalling the exp stream.
                    fqc, fh, foh_v = blk
                    tpf = ps_s.tile([128, 512], F32, tag="sg")
                    tp = tpf[:, 0:256].bitcast(BF16)
                    for s4 in range(4):
                        nc.tensor.matmul(
                            tp[:, s4 * 128:(s4 + 1) * 128],
                            foh_v[:, s4, :], ident[:],
                            is_transpose=True, start=True, stop=True,
                        )
                    otc = OT_v[:, fh, fqc * 512:(fqc + 1) * 512]
                    nc.vector.tensor_copy(otc, tp[:])
                    if fh == 0:
                        po = ps_o.tile([128, 512], F32, tag="po")
                        po_cur[0] = po
                        po_cur[1] = po[:].rearrange("p (c d) -> p c d", d=128)
                    po = po_cur[0]
                    for c4 in range(4):
                        c = fqc * 4 + c4
                        # start/stop are bank-granular: only the first/last
                        # matmul touching the bank may carry them.
                        nc.tensor.matmul(
                            po[:, c4 * 128:(c4 + 1) * 128],
                            OT_v[:, fh, c * 128:(c + 1) * 128],
                            wo_v[:, fh, :],
                            start=(fh == 0 and c4 == 0),
                            stop=(fh == H - 1 and c4 == 3),
                        )

                tasks = [(qc, h, g) for qc in range(NCS) for h in range(H)
                         for g in range(NGRP)]
                pend = emit_s(*tasks[0])
                deferred = None
                for i, (qc, h, g) in enumerate(tasks):
                    sgs = pend
                    is_dve = g in DVE_Q[qc]
                    mixed = g == SWING_G and (qc, h) in SWING
                    if is_dve:
                        if LAST_HEAD_ACT and qc == 1 and h == H - 1:
                            # last head: DVE is winding down; ACT (idle at the
                            # tail) does the exp, still bf16 for the bf16 AV
                            pt16 = pp.tile([128, NG * 512], BF16, tag="pt16")
                            pt16_v = pt16[:].rearrange(
                                "p (c n) -> p c n", n=512)
                            for ec in range(NG):
                                nc.scalar.activation(
                                    pt16_v[:, ec, :], sgs[ec][:], AF.Exp,
                                    bias=expb[:], scale=SCALE)
                            praw_v = pt16_v
                        else:
                            pti = pp.tile([128, NG * 512], I16, tag="pti")
                            for ec in range(NG):
                                nc.vector.tensor_scalar(
                                    pti[:, ec * 512:(ec + 1) * 512],
                                    sgs[ec][:], SCH_MUL, SCH_ADD,
                                    ALU.mult, ALU.add)
                            praw_v = pti[:].bitcast(BF16).rearrange(
                                "p (c n) -> p c n", n=512)
                    elif mixed:
                        pt8 = pp.tile([128, NG * 512], F8, tag="pt8")
                        pt8_v = pt8[:].rearrange("p (c n) -> p c n", n=512)
                        nc.scalar.activation(pt8_v[:, 0, :], sgs[0][:],
                                             AF.Exp, bias=expb[:],
                                             scale=SCALE)
                        pti = pp.tile([128, NG * 512], I16, tag="pti")
                        nc.vector.tensor_scalar(pti[:, 0:512], sgs[1][:],
                                                SCH_MUL, SCH_ADD,
                                                ALU.mult, ALU.add)
                        praw1 = pti[:, 0:512].bitcast(BF16)
                    else:
                        pt8 = pp.tile([128, NG * 512], F8, tag="pt8")
                        pt8_v = pt8[:].rearrange("p (c n) -> p c n", n=512)
                        for ec in range(NG):
                            nc.scalar.activation(pt8_v[:, ec, :], sgs[ec][:],
                                                 AF.Exp, bias=expb[:],
                                                 scale=SCALE)
                    # PE lookahead: next S group, then interleaved proj work
                    if i + 1 < len(tasks):
                        pend = emit_s(*tasks[i + 1])
                    if g == 7 and deferred is not None:
                        flush_block(deferred)
                        deferred = None
                        if qc == 1 and h == 0:
                            # half 0 is fully projected now; finalize it while
                            # qc1 attention streams (piece 2 two tasks later
                            # so the DVE chain interleaves with Schraudolphs)
                            ln_piece(0, po_cur[1], 0)
                            if not LN_STAGGER:
                                for p2 in range(1, LN_PIECES):
                                    ln_piece(0, po_cur[1], p2)
                    if (LN_STAGGER and qc == 1 and h == 0 and g == 5
                            and LN_PIECES > 1):
                        ln_piece(0, po_cur[1], 1)
                    if qc == 0:
                        if h + 1 < H and g == 3:
                            emit_rq(h + 1)
                    if g == 0:
                        avA = ps_av.tile([128, 512], F32, tag="avA")
                        avB = ps_av.tile([128, 512], F32, tag="avB")
                        oh = hp.tile([128, 4 * 128], BF16, tag="oh")
                        oh_v = oh[:].rearrange("p (s d) -> p s d", d=128)
                    if is_dve:
                        # c-outer: the first chunk's four AVs need only the
                        # first Schraudolph half
                        for c in range(NG):
                            mc = g * NG + c
                            for s in range(4):
                                av = avA if s < 2 else avB
                                off = (s % 2) * 256
                                nc.tensor.matmul(
                                    av[:, off:off + 129],
                                    praw_v[:, c, s * 128:(s + 1) * 128],
                                    xb16_v[:, VB_SLOT[mc], 0:129],
                                    start=(g == 0 and s % 2 == 0 and c == 0),
                                    stop=(g == NGRP - 1 and s % 2 == 1
                                          and c == NG - 1),
                                )
                    elif mixed:
                        for s in range(4):
                            av = avA if s < 2 else avB
                            off = (s % 2) * 256
                            nc.tensor.matmul(
                                av[:, off:off + 129],
                                pt8_v[:, 0, s * 128:(s + 1) * 128],
                                xa8_v[:, VA_SLOT[g * NG], 0:129],
                                start=(g == 0 and s % 2 == 0),
                                stop=False,
                            )
                        for s in range(4):
                            av = avA if s < 2 else avB
                            off = (s % 2) * 256
                            nc.tensor.matmul(
                                av[:, off:off + 129],
                                praw1[:, s * 128:(s + 1) * 128],
                                xb16_v[:, VB_SLOT[g * NG + 1], 0:129],
                                start=False,
                                stop=(g == NGRP - 1 and s % 2 == 1),
                            )
                    else:
                        for s in range(4):
                            av = avA if s < 2 else avB
                            off = (s % 2) * 256
                            slot = VA_SLOT[g * NG]
                            nc.tensor.matmul(
                                av[:, off:off + 129],
                                pt8_v[:, :, s * 128:(s + 1) * 128],
                                xa8_v[:, slot:slot + 2, 0:129],
                                start=(g == 0 and s % 2 == 0),
                                stop=(g == NGRP - 1 and s % 2 == 1),
                                perf_mode=DR,
                            )
                    if g == NGRP - 1:
                        # normalize now; transposes are deferred. qc0: DVE
                        # (ACT is carrying the projection copies); qc1: ACT
                        # scale-activation (DVE carries 4 exp groups there)
                        for t_i, av in ((0, avA), (1, avB)):
                            av_v = av[:].rearrange("p (r q) -> p r q", q=256)
                            rec2 = sp.tile([128, 2], F32, tag="rec")
                            nc.vector.reciprocal(rec2[:].unsqueeze(-1),
                                                 av_v[:, :, 128:129])
                            if qc == 0 or not QC1_NORM_ACT:
                                nc.vector.tensor_tensor(
                                    oh_v[:, 2 * t_i:2 * t_i + 2, :],
                                    av_v[:, :, 0:128],
                                    rec2[:].unsqueeze(-1).broadcast_to(
                                        [128, 2, 128]),
                                    ALU.mult)
                            else:
                                for k2 in range(2):
                                    nc.scalar.activation(
                                        oh_v[:, 2 * t_i + k2, :],
                                        av_v[:, k2, 0:128], AF.Identity,
                                        scale=rec2[:, k2:k2 + 1])
                        deferred = (qc, h, oh_v)
                flush_block(deferred)
                ln_half(1, po_cur[0], po_cur[1])

    nc.compile()
    return nc


@lru_cache(maxsize=2)
def _program(affine: bool = False):
    return _build_program(affine)


class _Executor:
    """Caches the jitted shard_map executable across kernel() calls."""

    def __init__(self, nc):
        import jax
        import concourse.mybir as mb
        from concourse import bass2jax
        from jax.sharding import Mesh, PartitionSpec
        from jax.experimental.shard_map import shard_map

        bass2jax.install_neuronx_cc_hook()
        self.jax = jax
        partition_name = (
            nc.partition_id_tensor.name if nc.partition_id_tensor else None
        )
        in_names, out_names, out_avals, zero_shapes = [], [], [], []
        for alloc in nc.m.functions[0].allocations:
            if not isinstance(alloc, mb.MemoryLocationSet):
                continue
            name = alloc.memorylocations[0].name
            if alloc.kind == "ExternalInput":
                if name != partition_name:
                    in_names.append(name)
            elif alloc.kind == "ExternalOutput":
                out_names.append(name)
                shape = tuple(alloc.tensor_shape)
                dtype = mb.dt.np(alloc.dtype)
                out_avals.append(jax.core.ShapedArray(shape, dtype))
                zero_shapes.append((shape, dtype))
        self.n_params = len(in_names)
        self.in_names = list(in_names)
        self.out_names = out_names
        self.out_avals = out_avals
        self.zero_shapes = zero_shapes
        all_in = in_names + out_names + ([partition_name] if partition_name else [])
        donate = tuple(range(self.n_params, self.n_params + len(out_names)))

        def _body(*args):
            operands = list(args)
            if partition_name is not None:
                operands.append(bass2jax.partition_id_tensor())
            return tuple(bass2jax._bass_exec_p.bind(
                *operands,
                out_avals=tuple(out_avals),
                in_names=tuple(all_in),
                out_names=tuple(out_names),
                lowering_input_output_aliases=(),
                sim_require_finite=True,
                sim_require_nnan=True,
                nc=nc,
            ))

        devices = jax.devices()[:NCORES]
        mesh = Mesh(np.asarray(devices), ("core",))
        n_in = self.n_params + len(out_names)
        self.sharded = jax.jit(
            shard_map(_body, mesh=mesh,
                      in_specs=(PartitionSpec("core"),) * n_in,
                      out_specs=(PartitionSpec("core"),) * len(out_names),
                      check_rep=False),
            donate_argnums=donate, keep_unused=True,
        )

    def concat_inputs(self, in_maps):
        return [
            np.concatenate([np.asarray(m[name]) for m in in_maps], axis=0)
            for name in self.in_names
        ]

    def zeros(self):
        return [np.zeros((NCORES * s[0], *s[1:]), d) for s, d in self.zero_shapes]

    def run(self, concat_in):
        out_arrs = self.sharded(*concat_in, *self.zeros())
        return out_arrs

    def split(self, out_arrs):
        return [
            {name: np.asarray(out_arrs[i]).reshape(NCORES, *self.out_avals[i].shape)[c]
             for i, name in enumerate(self.out_names)}
            for c in range(NCORES)
        ]


@lru_cache(maxsize=2)
def _executor(affine: bool = False):
    return _Executor(_program(affine))


def _prep_core_inputs(core, x, attn_mask, node_mask, Wq, Wk, Wv, Wo, bo,
                      gamma, beta):
    b, half = core // 2, core % 2
    rsl = slice(half * NQ, (half + 1) * NQ)
    xb = np.ascontiguousarray(x[b])
    m = {}
    xbT = np.ascontiguousarray(xb.T)
    m["xT8"] = xbT.astype(ml_dtypes.float8_e4m3)
    # row-major x chunks with a ones column: A@x replaces A@V (Wv folded
    # into the output projection), and the ones give the softmax row-sums
    xR = np.ones((128, NMC, 130), np.float32)
    xR[:, :, 0:128] = xb.reshape(NMC, 128, F).transpose(1, 0, 2)
    xR[:, :, 129] = 0.0
    m["xa8"] = np.ascontiguousarray(
        xR[:, ACT_CHUNKS, :].reshape(128, -1)).astype(ml_dtypes.float8_e4m3)
    m["xb16"] = np.ascontiguousarray(
        xR[:, DVE_CHUNKS, :].reshape(128, -1)).astype(ml_dtypes.bfloat16)
    m["xqT"] = np.ascontiguousarray(xb[rsl].T).astype(ml_dtypes.bfloat16)
    m["xres"] = np.ascontiguousarray(
        (xb[rsl] + bo).reshape(8, 128, 128).transpose(1, 0, 2)
    )
    mT = np.where(attn_mask[b].T[:, rsl], 0.0, -240.0).astype(np.float32)
    m["maskb"] = np.ascontiguousarray(
        mT.reshape(NMC, 128, NQ).transpose(1, 0, 2)
    ).astype(ml_dtypes.float8_e4m3)
    # fold K/Q projections: mT slab h = Wq_h @ Wk_h^T, so that on device
    # rq_h = mT_h^T @ xq^T = (Wk_h Wq_h^T) xq^T and S_h = x^T . rq_h
    wq3 = Wq.reshape(F, H, D).transpose(1, 0, 2)   # [h, f, d]
    wk3 = Wk.reshape(F, H, D).transpose(1, 0, 2)
    mT3 = np.einsum("hfd,hgd->hfg", wq3, wk3)      # [h, f, g]
    m["mT"] = np.ascontiguousarray(
        mT3.transpose(1, 0, 2)).astype(ml_dtypes.bfloat16)  # [f, h, g]

    # fold Wv into the output projection: wvo_h = Wv_h @ Wo_h
    wv3 = Wv.reshape(F, H, D).transpose(1, 0, 2)          # [h, f, d]
    wo3 = Wo.reshape(H, D, 128)                           # [h, d, o]
    wvo3 = np.einsum("hfd,hdo->hfo", wv3, wo3)            # [h, f, o]
    m["wvo"] = np.ascontiguousarray(
        wvo3.transpose(1, 0, 2)).astype(ml_dtypes.bfloat16)  # [f, h, o]
    m["gb"] = np.ascontiguousarray(np.stack([gamma, beta]))
    m["nm"] = np.ascontiguousarray(
        node_mask[b, rsl].astype(np.float32).reshape(8, 128).T
    )
    return m


def kernel(x, attn_mask, node_mask, Wq, Wk, Wv, Wo, bo, gamma, beta):
    x = np.asarray(x, np.float32)
    attn_mask = np.asarray(attn_mask, bool)
    node_mask = np.asarray(node_mask, bool)
    Wq = np.ascontiguousarray(np.asarray(Wq, np.float32))
    Wk = np.ascontiguousarray(np.asarray(Wk, np.float32))
    Wv = np.ascontiguousarray(np.asarray(Wv, np.float32))
    Wo = np.asarray(Wo, np.float32)
    bo = np.asarray(bo, np.float32)
    gamma = np.asarray(gamma, np.float32)
    beta = np.asarray(beta, np.float32)

    affine = not (np.all(gamma == 1.0) and np.all(beta == 0.0))
    ex = _executor(affine)
    in_maps = [
        _prep_core_inputs(c, x, attn_mask, node_mask, Wq, Wk, Wv, Wo, bo,
                          gamma, beta)
        for c in range(NCORES)
    ]
    results = ex.split(ex.run(ex.concat_inputs(in_maps)))
    out = np.empty((B, N, D), np.float32)
    for core in range(NCORES):
        b, half = core // 2, core % 2
        o = results[core]["out"]  # [128, 8, 128]
        out[b, half * NQ:(half + 1) * NQ] = (
            o.transpose(1, 0, 2).reshape(NQ, 128)
        )
    return out

